# revision 12
# baseline (speedup 1.0000x reference)
"""GPT2 paged-attention decode kernel for Trainium2 (Bass/Tile), 8-core SPMD.

Problem: B=32 batches, Q=1 query, D=1024, H=16 heads, DH=64, KV cache 8192.
  qkv = hidden @ w_attn + b_attn; split into q, k_cur, v_cur
  attention over concat(cache, current) per (b, h)  [no scaling, no mask]
  out = (softmax contexts) @ w_proj + b_proj

Sharding: pure batch data-parallel, 4 batches per core, zero collectives.

Key optimization vs the fp32 version: K/V caches (and weights/activations)
are cast to fp16 on the HOST, halving device HBM traffic (256MB -> 128MB per
core) and unlocking the fast 16-bit paths on every engine:
  - DVE tensor_tensor runs in 2x_1p mode (0.5 cyc/elem)
  - TensorE matmul streams 1 col/cycle (fp32 is 4 cyc/col)
Precision: products/partial sums in fp16 give |err(s)| ~ 5e-3 on scores of
magnitude ~20 -> ~0.5% weight error, far inside the 2e-2 gate.

Softmax max-subtraction is dropped (shift invariance; fp32/bf16 exp cannot
overflow for |s| <~ 30: e^30 ~ 1e13 << bf16/fp32 max). exp outputs are
stored bf16 (range to 3e38) for the fast matmul path; denominators are
accumulated in fp32 via the activation accum_out.

Per (b,h) pair on-chip schedule:
  - K/V fp16 tiles [128, 64, 64] DMA'd in natural row layout (partition p
    holds keys p*64..p*64+63 contiguously; 1MB per DMA).
  - VectorE: kq = K * q_broadcast (fp16 2x); 3-level binary-tree adds over
    the head dim (fp16 2x); final 8->1 reduce_sum to fp32 s [128, 64].
  - ScalarE: e = exp(s) -> bf16 with accum_out giving per-partition
    denominator partials (fp32).
  - TensorE: ctx row [1,64] accumulated in PSUM via 64 matmuls, e columns
    stationary (1-col LDWEIGHTS), V tiles streaming; denominators reduced
    across partitions with a ones-matmul into a shared [1,64] PSUM row;
    ctx rows transposed into columns of ctx^T via rank-1 matmuls.
  - The current token never enters the stream: it is folded in a small
    epilogue over all 64 pairs at once (PE transposes of the qkv rows,
    one elementwise mul + ones-matmul partition reduce + exp).
"""

import os
import sys

import numpy as np

sys.path.insert(0, "/opt/trn_rl_repo")

import concourse.bass as bass
import concourse.tile as tile
from concourse import bacc, mybir
from concourse.bass_utils import run_bass_kernel_spmd

FP32 = mybir.dt.float32
FP16 = mybir.dt.float16
BF16 = mybir.dt.bfloat16

# Problem shape (hardcoded per contest rules).
B, D, H, DH, KV = 32, 1024, 16, 64, 8192
NCORES = 8
BC = B // NCORES          # batches per core = 4
NPAIRS = BC * H           # 64 (b,h) pairs per core
JT = KV // 128            # 64 key-slots per partition


def _bcast_ap(t_ap, col0, ncols, nparts, row_stride):
    """DRAM AP [nparts, rows, ncols] with partition stride 0 (broadcast)."""
    return bass.AP(
        tensor=t_ap.tensor,
        offset=t_ap.offset + col0,
        ap=[[0, nparts], [row_stride, t_ap.shape[0]], [1, ncols]],
    )


def build_kernel():
    nc = bacc.Bacc(
        "TRN2",
        target_bir_lowering=False,
        debug=False,
        enable_asserts=False,
        num_devices=NCORES,
    )

    hidden = nc.dram_tensor("hidden", [BC, D], FP16, kind="ExternalInput")
    k_cache = nc.dram_tensor("k_cache", [BC, H, KV, DH], FP16, kind="ExternalInput")
    v_cache = nc.dram_tensor("v_cache", [BC, H, KV, DH], FP16, kind="ExternalInput")
    # host-prepped: wa_prep[nb, p, t, n] = w_attn[t*128+p, nb*512+n]  (fp16)
    w_attn = nc.dram_tensor("w_attn", [6, 128, 8, 512], FP16, kind="ExternalInput")
    b_attn = nc.dram_tensor("b_attn", [3 * D], FP32, kind="ExternalInput")
    # host-prepped: wp_prep[p, h, n] = w_proj[h*64+p, n]  (fp16)
    w_proj = nc.dram_tensor("w_proj", [64, H, D], FP16, kind="ExternalInput")
    b_proj = nc.dram_tensor("b_proj", [D], FP32, kind="ExternalInput")
    ident = nc.dram_tensor("ident", [128, 128], FP16, kind="ExternalInput")
    out = nc.dram_tensor("out", [BC, D], FP32, kind="ExternalOutput")

    with tile.TileContext(nc) as tc:
        with (
            tc.tile_pool(name="singles", bufs=1) as singles,
            tc.tile_pool(name="wa_pool", bufs=2) as wa_pool,
            tc.tile_pool(name="kv_pool", bufs=4) as kv_pool,
            tc.tile_pool(name="kq_pool", bufs=2) as kq_pool,
            tc.tile_pool(name="se_pool", bufs=3) as se_pool,
            tc.tile_pool(name="dram_pool", bufs=1, space="DRAM") as dram_pool,
            tc.tile_pool(name="mm512", bufs=2, space="PSUM") as mm512,
            tc.tile_pool(name="ctx_ps", bufs=3, space="PSUM") as ctx_ps,
            tc.tile_pool(name="ctxt_ps", bufs=1, space="PSUM") as ctxt_ps,
            tc.tile_pool(name="l_ps", bufs=1, space="PSUM") as l_ps,
        ):
            # ---- constants ----
            ones32 = singles.tile([128, 64], FP32)
            nc.vector.memset(ones32, 1.0)
            ones16 = singles.tile([128, 64], FP16)
            nc.vector.memset(ones16, 1.0)
            id_sb = singles.tile([128, 128], FP16)
            nc.gpsimd.dma_start(out=id_sb, in_=ident.ap())

            ba_sb = singles.tile([BC, 3 * D], FP32)
            nc.gpsimd.dma_start(
                out=ba_sb, in_=_bcast_ap(b_attn.ap().unsqueeze(0), 0, 3 * D, BC, 0)
            )
            bp_sb = singles.tile([BC, D], FP32)
            nc.gpsimd.dma_start(
                out=bp_sb, in_=_bcast_ap(b_proj.ap().unsqueeze(0), 0, D, BC, 0)
            )

            # ---- hidden -> hT via PE transposes ----
            hid_sb = singles.tile([BC, D], FP16)
            nc.gpsimd.dma_start(out=hid_sb, in_=hidden.ap())
            ps_hT = mm512.tile([128, 8, BC], FP16, tag="mm512", name="ps_hT")
            for t in range(8):
                nc.tensor.transpose(
                    ps_hT[:, t, :],
                    hid_sb[:, t * 128 : (t + 1) * 128],
                    id_sb[0:BC, 0:BC],
                )
            hT = singles.tile([128, 8, BC], FP16)
            nc.scalar.copy(hT, ps_hT)

            # ---- w_proj ----
            wp_sb = singles.tile([64, H, D], FP16)
            nc.gpsimd.dma_start(out=wp_sb, in_=w_proj.ap())

            # ---- QKV projection: qkv = hidden @ w_attn + b_attn ----
            # q third first (unblocks the attention stream), then k/v thirds.
            qkv_sb = singles.tile([BC, 3 * D], FP16)
            q_dram = dram_pool.tile([BC, D], FP16)
            for nb in (0, 1, 2, 3, 4, 5):
                wa_t = wa_pool.tile([128, 8, 512], FP16, tag="wa")
                nc.gpsimd.dma_start(out=wa_t, in_=w_attn.ap()[nb])
                ps = mm512.tile([BC, 512], FP32, tag="mm512")
                for t in range(8):
                    nc.tensor.matmul(
                        ps,
                        hT[:, t, :],
                        wa_t[:, t, :],
                        start=(t == 0),
                        stop=(t == 7),
                    )
                nc.vector.tensor_add(
                    qkv_sb[:, nb * 512 : (nb + 1) * 512],
                    ps,
                    ba_sb[:, nb * 512 : (nb + 1) * 512],
                )
                if nb == 1:
                    # q complete -> round-trip through DRAM for the
                    # partition-broadcast load
                    nc.sync.dma_start(out=q_dram, in_=qkv_sb[:, 0:D])

            # qb_all[p, b, :] = q row of batch b (same for every partition p)
            qb_all = singles.tile([128, BC, D], FP16)
            nc.scalar.dma_start(out=qb_all, in_=_bcast_ap(q_dram, 0, D, 128, D))

            # ---- attention stream over pairs (h-major: j = h*BC + b) ----
            psum_ctxT = ctxt_ps.tile([64, NPAIRS], FP32)  # ctx^T, one col per pair
            psum_l = l_ps.tile([1, NPAIRS], FP32)         # softmax denominators

            for j in range(NPAIRS):
                h, b = j // BC, j % BC

                k_sb = kv_pool.tile([128, JT, DH], FP16, tag="k")
                v_sb = kv_pool.tile([128, JT, DH], FP16, tag="v")
                nc.sync.dma_start(
                    out=k_sb,
                    in_=k_cache.ap()[b, h].rearrange("(p jj) d -> p jj d", jj=JT),
                )
                nc.scalar.dma_start(
                    out=v_sb,
                    in_=v_cache.ap()[b, h].rearrange("(p jj) d -> p jj d", jj=JT),
                )

                # s[p, jj] = sum_d K[p, jj, d] * q[d]   (fp16 2x tree)
                kq = kq_pool.tile([128, JT, DH], FP16, tag="kq")
                qb = qb_all[:, b, h * DH : (h + 1) * DH].unsqueeze(1).broadcast_to(
                    [128, JT, DH]
                )
                nc.vector.tensor_mul(kq, k_sb, qb)
                t32 = kq_pool.tile([128, JT, 32], FP16, tag="t32")
                nc.vector.tensor_add(t32, kq[:, :, 0:32], kq[:, :, 32:64])
                t16 = kq_pool.tile([128, JT, 16], FP16, tag="t16")
                nc.vector.tensor_add(t16, t32[:, :, 0:16], t32[:, :, 16:32])
                t8 = kq_pool.tile([128, JT, 8], FP16, tag="t8")
                nc.vector.tensor_add(t8, t16[:, :, 0:8], t16[:, :, 8:16])
                s_sb = se_pool.tile([128, JT], FP32, tag="s")
                nc.vector.reduce_sum(s_sb, t8, axis=mybir.AxisListType.X)

                # e = exp(s) in bf16; lp = per-partition denominator partials
                e_sb = se_pool.tile([128, JT], BF16, tag="e")
                lp = se_pool.tile([128, 1], FP32, tag="lp")
                nc.scalar.activation(
                    e_sb,
                    s_sb,
                    mybir.ActivationFunctionType.Exp,
                    accum_out=lp,
                )

                # ctx row [1, dh]: e column stationary (1-col LDWEIGHTS),
                # V tile streams 64 fp16 cols/matmul.
                psum_row = ctx_ps.tile([1, DH], FP32, tag="ctx")
                for jj in range(JT):
                    nc.tensor.matmul(
                        psum_row,
                        e_sb[:, jj : jj + 1],
                        v_sb[:, jj, :],
                        start=(jj == 0),
                        stop=(jj == JT - 1),
                    )
                # denominator: l[j] = sum_p lp[p]
                nc.tensor.matmul(
                    psum_l[:, j : j + 1],
                    lp,
                    ones32[:, 0:1],
                    start=True,
                    stop=True,
                    skip_group_check=True,
                )
                # transpose the row into column j of ctx^T via a rank-1 matmul
                ctx_row = se_pool.tile([1, DH], FP32, tag="ctxrow")
                nc.scalar.copy(ctx_row, psum_row)
                nc.tensor.matmul(
                    psum_ctxT[:, j : j + 1],
                    ctx_row,
                    ones32[0:1, 0:1],
                    start=True,
                    stop=True,
                    skip_group_check=True,
                )

            # ---- epilogue: fold in the current token for all pairs ----
            # Transpose q/k_cur/v_cur rows [4, 64] -> [64, (h, b)] tiles.
            ps_tr = mm512.tile([64, 3, H, BC], FP16, tag="mm512", name="ps_tr")
            for part in range(3):
                for h in range(H):
                    nc.tensor.transpose(
                        ps_tr[:, part, h, :],
                        qkv_sb[:, part * D + h * DH : part * D + (h + 1) * DH],
                        id_sb[0:BC, 0:BC],
                    )
            qT = singles.tile([64, H, BC], FP16)
            nc.scalar.copy(qT, ps_tr[:, 0])
            kT = singles.tile([64, H, BC], FP16)
            nc.scalar.copy(kT, ps_tr[:, 1])
            vT = singles.tile([64, H, BC], FP32)
            nc.vector.tensor_copy(vT, ps_tr[:, 2])

            # s_cur[j] = sum_d q[d, j] * k_cur[d, j]  (partition reduce by MM)
            qk = singles.tile([64, H, BC], FP16)
            nc.vector.tensor_mul(qk, qT, kT)
            ps_scur = ctx_ps.tile([1, NPAIRS], FP32, tag="ctx", name="ps_scur")
            nc.tensor.matmul(ps_scur, ones16[0:64, 0:1], qk, start=True, stop=True)
            e_cur = singles.tile([1, NPAIRS], FP32)
            nc.scalar.activation(e_cur, ps_scur, mybir.ActivationFunctionType.Exp)
            # add e_cur into the denominators (SBUF side: a matmul with
            # start=False would lose columns whose has_written bits were
            # reset by later groups' start in the same PSUM bank)
            l_tot = singles.tile([1, NPAIRS], FP32)
            nc.vector.tensor_add(l_tot, psum_l, e_cur)
            # broadcast e_cur across 64 partitions (rank-1 ones outer product)
            ps_eb = mm512.tile([64, NPAIRS], FP32, tag="mm512", name="ps_eb")
            nc.tensor.matmul(ps_eb, ones32[0:1, 0:64], e_cur, start=True, stop=True)
            eb_sb = singles.tile([64, NPAIRS], FP32)
            nc.scalar.copy(eb_sb, ps_eb)
            vc = singles.tile([64, NPAIRS], FP32)
            nc.vector.tensor_mul(vc, vT, eb_sb)
            ctx_full = singles.tile([64, NPAIRS], FP32)
            nc.vector.tensor_add(ctx_full, psum_ctxT, vc)

            # ---- divide by denominators ----
            r_sb = singles.tile([1, NPAIRS], FP32)
            nc.vector.reciprocal(r_sb, l_tot)
            ps_rb = mm512.tile([64, NPAIRS], FP32, tag="mm512", name="ps_rb")
            nc.tensor.matmul(ps_rb, ones32[0:1, 0:64], r_sb, start=True, stop=True)
            rb_sb = singles.tile([64, NPAIRS], FP32)
            nc.scalar.copy(rb_sb, ps_rb)
            ctx_scaled = singles.tile([64, NPAIRS], FP16)
            nc.vector.tensor_mul(ctx_scaled, ctx_full, rb_sb)

            # ---- c_proj: out[b,:] = sum_h ctx[b,h,:] @ w_proj[h*64:(h+1)*64, :] ----
            ps_o = [
                mm512.tile([BC, 512], FP32, tag="mm512", name=f"ps_o{i}")
                for i in range(2)
            ]
            for h in range(H):
                for nb in range(2):
                    nc.tensor.matmul(
                        ps_o[nb],
                        ctx_scaled[:, h * BC : (h + 1) * BC],
                        wp_sb[:, h, nb * 512 : (nb + 1) * 512],
                        start=(h == 0),
                        stop=(h == H - 1),
                    )
            out_sb = singles.tile([BC, D], FP32)
            for nb in range(2):
                nc.vector.tensor_add(
                    out_sb[:, nb * 512 : (nb + 1) * 512],
                    ps_o[nb],
                    bp_sb[:, nb * 512 : (nb + 1) * 512],
                )
            nc.sync.dma_start(out=out.ap(), in_=out_sb)

    nc.compile()
    return nc


_NC_CACHE = None


def _get_nc():
    global _NC_CACHE
    if _NC_CACHE is None:
        _NC_CACHE = build_kernel()
    return _NC_CACHE


def make_in_maps(inputs):
    """Shard full inputs into per-core input maps (batch data-parallel).

    Heavy tensors are cast to fp16 host-side: the device kernel streams
    fp16, halving HBM traffic.
    """
    hidden = np.asarray(inputs["hidden_states"], np.float32).reshape(B, D)
    hidden16 = np.ascontiguousarray(hidden.astype(np.float16))
    k16 = np.asarray(inputs["k_cache"]).astype(np.float16)
    v16 = np.asarray(inputs["v_cache"]).astype(np.float16)
    wa = np.asarray(inputs["w_attn"], np.float32).astype(np.float16)
    # wa_prep[nb, p, t, n] = w_attn[t*128+p, nb*512+n]
    wa_prep = np.ascontiguousarray(
        wa.reshape(8, 128, 6, 512).transpose(2, 1, 0, 3)
    )
    wp = np.asarray(inputs["w_proj"], np.float32).astype(np.float16)
    # wp_prep[p, h, n] = w_proj[h*64+p, n]
    wp_prep = np.ascontiguousarray(wp.reshape(H, 64, D).transpose(1, 0, 2))
    b_attn = np.ascontiguousarray(np.asarray(inputs["b_attn"], np.float32))
    b_proj = np.ascontiguousarray(np.asarray(inputs["b_proj"], np.float32))
    ident = np.eye(128, dtype=np.float16)
    in_maps = []
    for c in range(NCORES):
        sl = slice(c * BC, (c + 1) * BC)
        in_maps.append(
            {
                "hidden": np.ascontiguousarray(hidden16[sl]),
                "k_cache": np.ascontiguousarray(k16[sl]),
                "v_cache": np.ascontiguousarray(v16[sl]),
                "w_attn": wa_prep,
                "b_attn": b_attn,
                "w_proj": wp_prep,
                "b_proj": b_proj,
                "ident": ident,
            }
        )
    return in_maps


def kernel(**inputs):
    nc = _get_nc()
    in_maps = make_in_maps(inputs)
    res = run_bass_kernel_spmd(nc, in_maps, core_ids=list(range(NCORES)))
    out = np.empty((B, 1, D), np.float32)
    for c in range(NCORES):
        out[c * BC : (c + 1) * BC, 0, :] = np.asarray(res.results[c]["out"])
    return out


if __name__ == "__main__":
    build_kernel()
    print("kernel built OK")


# revision 19
# speedup vs baseline: 1.0798x; 1.0798x over previous
"""GPT2 paged-attention decode kernel for Trainium2 (Bass/Tile), 8-core SPMD.

Problem: B=32 batches, Q=1 query, D=1024, H=16 heads, DH=64, KV cache 8192.
  qkv = hidden @ w_attn + b_attn; split into q, k_cur, v_cur
  attention over concat(cache, current) per (b, h)  [no scaling, no mask]
  out = (softmax contexts) @ w_proj + b_proj

Sharding: pure batch data-parallel, 4 batches per core, zero collectives.

Key optimization vs the fp32 version: K/V caches (and weights/activations)
are cast to fp16 on the HOST, halving device HBM traffic (256MB -> 128MB per
core) and unlocking the fast 16-bit paths on every engine:
  - DVE tensor_tensor runs in 2x_1p mode (0.5 cyc/elem)
  - TensorE matmul streams 1 col/cycle (fp32 is 4 cyc/col)
Precision: products/partial sums in fp16 give |err(s)| ~ 5e-3 on scores of
magnitude ~20 -> ~0.5% weight error, far inside the 2e-2 gate.

Softmax max-subtraction is dropped (shift invariance; fp32/bf16 exp cannot
overflow for |s| <~ 30: e^30 ~ 1e13 << bf16/fp32 max). exp outputs are
stored bf16 (range to 3e38) for the fast matmul path; denominators are
accumulated in fp32 via the activation accum_out.

Per (b,h) pair on-chip schedule:
  - K/V fp16 tiles [128, 64, 64] DMA'd in natural row layout (partition p
    holds keys p*64..p*64+63 contiguously; 1MB per DMA).
  - VectorE: kq = K * q_broadcast (fp16 2x); 3-level binary-tree adds over
    the head dim (fp16 2x); final 8->1 reduce_sum to fp32 s [128, 64].
  - ScalarE: e = exp(s) -> bf16 with accum_out giving per-partition
    denominator partials (fp32).
  - TensorE: ctx row [1,64] accumulated in PSUM via 64 matmuls, e columns
    stationary (1-col LDWEIGHTS), V tiles streaming; denominators reduced
    across partitions with a ones-matmul into a shared [1,64] PSUM row;
    ctx rows transposed into columns of ctx^T via rank-1 matmuls.
  - The current token never enters the stream: it is folded in a small
    epilogue over all 64 pairs at once (PE transposes of the qkv rows,
    one elementwise mul + ones-matmul partition reduce + exp).
"""

import os
import sys

import numpy as np

sys.path.insert(0, "/opt/trn_rl_repo")

import concourse.bass as bass
import concourse.tile as tile
from concourse import bacc, mybir
from concourse.bass_utils import run_bass_kernel_spmd

FP32 = mybir.dt.float32
FP16 = mybir.dt.float16
BF16 = mybir.dt.bfloat16

# Problem shape (hardcoded per contest rules).
B, D, H, DH, KV = 32, 1024, 16, 64, 8192
NCORES = 8
BC = B // NCORES          # batches per core = 4
NPAIRS = BC * H           # 64 (b,h) pairs per core
JT = KV // 128            # 64 key-slots per partition


def _bcast_ap(t_ap, col0, ncols, nparts, row_stride):
    """DRAM AP [nparts, rows, ncols] with partition stride 0 (broadcast)."""
    return bass.AP(
        tensor=t_ap.tensor,
        offset=t_ap.offset + col0,
        ap=[[0, nparts], [row_stride, t_ap.shape[0]], [1, ncols]],
    )


def build_kernel():
    nc = bacc.Bacc(
        "TRN2",
        target_bir_lowering=False,
        debug=False,
        enable_asserts=False,
        num_devices=NCORES,
    )

    hidden = nc.dram_tensor("hidden", [BC, D], FP16, kind="ExternalInput")
    k_cache = nc.dram_tensor("k_cache", [BC, H, KV, DH], FP16, kind="ExternalInput")
    v_cache = nc.dram_tensor("v_cache", [BC, H, KV, DH], FP16, kind="ExternalInput")
    # host-prepped: wa_prep[nb, p, t, n] = w_attn[t*128+p, nb*512+n]  (fp16)
    w_attn = nc.dram_tensor("w_attn", [6, 128, 8, 512], FP16, kind="ExternalInput")
    b_attn = nc.dram_tensor("b_attn", [3 * D], FP32, kind="ExternalInput")
    # host-prepped: wp_prep[p, h, n] = w_proj[h*64+p, n]  (fp16)
    w_proj = nc.dram_tensor("w_proj", [64, H, D], FP16, kind="ExternalInput")
    b_proj = nc.dram_tensor("b_proj", [D], FP32, kind="ExternalInput")
    ident = nc.dram_tensor("ident", [128, 128], FP16, kind="ExternalInput")
    out = nc.dram_tensor("out", [BC, D], FP32, kind="ExternalOutput")

    with tile.TileContext(nc) as tc:
        with (
            tc.tile_pool(name="singles", bufs=1) as singles,
            tc.tile_pool(name="wa_pool", bufs=2) as wa_pool,
            tc.tile_pool(name="kv_pool", bufs=2) as kv_pool,
            tc.tile_pool(name="kq_pool", bufs=2) as kq_pool,
            tc.tile_pool(name="se_pool", bufs=4) as se_pool,
            tc.tile_pool(name="dram_pool", bufs=1, space="DRAM") as dram_pool,
            tc.tile_pool(name="mm512", bufs=2, space="PSUM") as mm512,
            tc.tile_pool(name="ctx_ps", bufs=3, space="PSUM") as ctx_ps,
            tc.tile_pool(name="ctxt_ps", bufs=1, space="PSUM") as ctxt_ps,
            tc.tile_pool(name="l_ps", bufs=1, space="PSUM") as l_ps,
        ):
            # ---- constants ----
            ones32 = singles.tile([128, 64], FP32)
            nc.vector.memset(ones32, 1.0)
            ones16 = singles.tile([128, 64], FP16)
            nc.vector.memset(ones16, 1.0)
            id_sb = singles.tile([128, 128], FP16)
            nc.gpsimd.dma_start(out=id_sb, in_=ident.ap())

            ba_sb = singles.tile([BC, 3 * D], FP32)
            nc.gpsimd.dma_start(
                out=ba_sb, in_=_bcast_ap(b_attn.ap().unsqueeze(0), 0, 3 * D, BC, 0)
            )
            bp_sb = singles.tile([BC, D], FP32)
            nc.gpsimd.dma_start(
                out=bp_sb, in_=_bcast_ap(b_proj.ap().unsqueeze(0), 0, D, BC, 0)
            )

            # ---- hidden -> hT via PE transposes ----
            hid_sb = singles.tile([BC, D], FP16)
            nc.gpsimd.dma_start(out=hid_sb, in_=hidden.ap())
            ps_hT = mm512.tile([128, 8, BC], FP16, tag="mm512", name="ps_hT")
            for t in range(8):
                nc.tensor.transpose(
                    ps_hT[:, t, :],
                    hid_sb[:, t * 128 : (t + 1) * 128],
                    id_sb[0:BC, 0:BC],
                )
            hT = singles.tile([128, 8, BC], FP16)
            nc.scalar.copy(hT, ps_hT)

            # ---- w_proj ----
            wp_sb = singles.tile([64, H, D], FP16)
            nc.gpsimd.dma_start(out=wp_sb, in_=w_proj.ap())

            # ---- QKV projection: qkv = hidden @ w_attn + b_attn ----
            # q third first (unblocks the attention stream), then k/v thirds.
            qkv_sb = singles.tile([BC, 3 * D], FP16)
            q_dram = dram_pool.tile([BC, D], FP16)
            for nb in (0, 1, 2, 3, 4, 5):
                wa_t = wa_pool.tile([128, 8, 512], FP16, tag="wa")
                nc.gpsimd.dma_start(out=wa_t, in_=w_attn.ap()[nb])
                ps = mm512.tile([BC, 512], FP32, tag="mm512")
                for t in range(8):
                    nc.tensor.matmul(
                        ps,
                        hT[:, t, :],
                        wa_t[:, t, :],
                        start=(t == 0),
                        stop=(t == 7),
                    )
                nc.vector.tensor_add(
                    qkv_sb[:, nb * 512 : (nb + 1) * 512],
                    ps,
                    ba_sb[:, nb * 512 : (nb + 1) * 512],
                )
                if nb == 1:
                    # q complete -> round-trip through DRAM for the
                    # partition-broadcast load. Keep the sync/scalar HWDGE
                    # queues free for the K/V streams (FIFO head-of-line!).
                    nc.gpsimd.dma_start(out=q_dram, in_=qkv_sb[:, 0:D])

            # qb_all[p, b, :] = q row of batch b (same for every partition p)
            qb_all = singles.tile([128, BC, D], FP16)
            nc.gpsimd.dma_start(out=qb_all, in_=_bcast_ap(q_dram, 0, D, 128, D))

            # ---- stream-independent part of the current-token epilogue ----
            # (emitted early: PE transposes of the qkv rows only need qkv_sb)
            ps_tr = mm512.tile([64, 3, H, BC], FP16, tag="mm512", name="ps_tr")
            for part in range(3):
                for h in range(H):
                    nc.tensor.transpose(
                        ps_tr[:, part, h, :],
                        qkv_sb[:, part * D + h * DH : part * D + (h + 1) * DH],
                        id_sb[0:BC, 0:BC],
                    )
            qT = singles.tile([64, H, BC], FP16)
            nc.scalar.copy(qT, ps_tr[:, 0])
            kT = singles.tile([64, H, BC], FP16)
            nc.scalar.copy(kT, ps_tr[:, 1])
            vT = singles.tile([64, H, BC], FP32)
            nc.vector.tensor_copy(vT, ps_tr[:, 2])

            # s_cur[j] = sum_d q[d, j] * k_cur[d, j]  (partition reduce by MM)
            qk = singles.tile([64, H, BC], FP16)
            nc.vector.tensor_mul(qk, qT, kT)
            ps_scur = mm512.tile([1, NPAIRS], FP32, tag="mm512", name="ps_scur")
            nc.tensor.matmul(ps_scur, ones16[0:64, 0:1], qk, start=True, stop=True)
            e_cur = singles.tile([1, NPAIRS], FP32)
            nc.scalar.activation(e_cur, ps_scur, mybir.ActivationFunctionType.Exp)
            # broadcast e_cur across 64 partitions (rank-1 ones outer product)
            ps_eb = mm512.tile([64, NPAIRS], FP32, tag="mm512", name="ps_eb")
            nc.tensor.matmul(ps_eb, ones32[0:1, 0:64], e_cur, start=True, stop=True)
            eb_sb = singles.tile([64, NPAIRS], FP32)
            nc.scalar.copy(eb_sb, ps_eb)
            vc = singles.tile([64, NPAIRS], FP32)
            nc.vector.tensor_mul(vc, vT, eb_sb)

            # ---- attention stream over pairs (h-major: j = h*BC + b) ----
            psum_ctxT = ctxt_ps.tile([64, NPAIRS], FP32)  # ctx^T, one col per pair
            psum_l = l_ps.tile([1, NPAIRS], FP32)         # softmax denominators

            # Two (b, h) pairs per group (adjacent heads of one batch): DVE
            # ops process both pairs' tiles at once to amortize per-op cost.
            for g in range(NPAIRS // 2):
                hh, b = g // BC, g % BC
                h0 = 2 * hh

                k2 = kv_pool.tile([128, 2, JT, DH], FP16, tag="k")
                v2 = kv_pool.tile([128, 2, JT, DH], FP16, tag="v")
                for i in range(2):
                    nc.sync.dma_start(
                        out=k2[:, i],
                        in_=k_cache.ap()[b, h0 + i].rearrange(
                            "(p jj) d -> p jj d", jj=JT
                        ),
                    )
                    nc.scalar.dma_start(
                        out=v2[:, i],
                        in_=v_cache.ap()[b, h0 + i].rearrange(
                            "(p jj) d -> p jj d", jj=JT
                        ),
                    )

                # s[p, hh, jj] = sum_d K[p, hh, jj, d] * q_hh[d]
                # fp16 2x mul + in-place binary-tree halving adds
                kq = kq_pool.tile([128, 2, JT, DH], FP16, tag="kq")
                qb = (
                    qb_all[:, b, h0 * DH : (h0 + 2) * DH]
                    .rearrange("p (i d) -> p i d", i=2)
                    .unsqueeze(2)
                    .broadcast_to([128, 2, JT, DH])
                )
                nc.vector.tensor_mul(kq, k2, qb)
                nc.vector.tensor_add(
                    kq[:, :, :, 0:32], kq[:, :, :, 0:32], kq[:, :, :, 32:64]
                )
                nc.vector.tensor_add(
                    kq[:, :, :, 0:16], kq[:, :, :, 0:16], kq[:, :, :, 16:32]
                )
                nc.vector.tensor_add(
                    kq[:, :, :, 0:8], kq[:, :, :, 0:8], kq[:, :, :, 8:16]
                )
                s2 = se_pool.tile([128, 2, JT], FP32, tag="s")
                nc.vector.reduce_sum(s2, kq[:, :, :, 0:8], axis=mybir.AxisListType.X)

                # per pair: exp (bf16) + denominator partials, ctx matmuls
                e2 = se_pool.tile([128, 2, JT], BF16, tag="e")
                for i in range(2):
                    j = (h0 + i) * BC + b
                    lp = se_pool.tile([128, 1], FP32, tag="lp")
                    nc.scalar.activation(
                        e2[:, i],
                        s2[:, i],
                        mybir.ActivationFunctionType.Exp,
                        accum_out=lp,
                    )
                    psum_row = ctx_ps.tile([1, DH], FP32, tag="ctx")
                    for jj in range(JT):
                        nc.tensor.matmul(
                            psum_row,
                            e2[:, i, jj : jj + 1],
                            v2[:, i, jj, :],
                            start=(jj == 0),
                            stop=(jj == JT - 1),
                        )
                    # denominator: l[j] = sum_p lp[p]
                    nc.tensor.matmul(
                        psum_l[:, j : j + 1],
                        lp,
                        ones32[:, 0:1],
                        start=True,
                        stop=True,
                        skip_group_check=True,
                    )
                    # transpose the row into column j of ctx^T (rank-1 matmul)
                    ctx_row = se_pool.tile([1, DH], FP32, tag="ctxrow")
                    nc.scalar.copy(ctx_row, psum_row)
                    nc.tensor.matmul(
                        psum_ctxT[:, j : j + 1],
                        ctx_row,
                        ones32[0:1, 0:1],
                        start=True,
                        stop=True,
                        skip_group_check=True,
                    )

            # ---- epilogue: stream-dependent part ----
            # add e_cur into the denominators (SBUF side: a matmul with
            # start=False would lose columns whose has_written bits were
            # reset by later groups' start in the same PSUM bank)
            l_tot = singles.tile([1, NPAIRS], FP32)
            nc.vector.tensor_add(l_tot, psum_l, e_cur)
            ctx_full = singles.tile([64, NPAIRS], FP32)
            nc.vector.tensor_add(ctx_full, psum_ctxT, vc)

            # ---- divide by denominators ----
            r_sb = singles.tile([1, NPAIRS], FP32)
            nc.vector.reciprocal(r_sb, l_tot)
            ps_rb = mm512.tile([64, NPAIRS], FP32, tag="mm512", name="ps_rb")
            nc.tensor.matmul(ps_rb, ones32[0:1, 0:64], r_sb, start=True, stop=True)
            rb_sb = singles.tile([64, NPAIRS], FP32)
            nc.scalar.copy(rb_sb, ps_rb)
            ctx_scaled = singles.tile([64, NPAIRS], FP16)
            nc.vector.tensor_mul(ctx_scaled, ctx_full, rb_sb)

            # ---- c_proj: out[b,:] = sum_h ctx[b,h,:] @ w_proj[h*64:(h+1)*64, :] ----
            ps_o = [
                mm512.tile([BC, 512], FP32, tag="mm512", name=f"ps_o{i}")
                for i in range(2)
            ]
            for h in range(H):
                for nb in range(2):
                    nc.tensor.matmul(
                        ps_o[nb],
                        ctx_scaled[:, h * BC : (h + 1) * BC],
                        wp_sb[:, h, nb * 512 : (nb + 1) * 512],
                        start=(h == 0),
                        stop=(h == H - 1),
                    )
            out_sb = singles.tile([BC, D], FP32)
            for nb in range(2):
                nc.vector.tensor_add(
                    out_sb[:, nb * 512 : (nb + 1) * 512],
                    ps_o[nb],
                    bp_sb[:, nb * 512 : (nb + 1) * 512],
                )
            nc.gpsimd.dma_start(out=out.ap(), in_=out_sb)

    nc.compile()
    return nc


_NC_CACHE = None


def _get_nc():
    global _NC_CACHE
    if _NC_CACHE is None:
        _NC_CACHE = build_kernel()
    return _NC_CACHE


def make_in_maps(inputs):
    """Shard full inputs into per-core input maps (batch data-parallel).

    Heavy tensors are cast to fp16 host-side: the device kernel streams
    fp16, halving HBM traffic.
    """
    hidden = np.asarray(inputs["hidden_states"], np.float32).reshape(B, D)
    hidden16 = np.ascontiguousarray(hidden.astype(np.float16))
    k16 = np.asarray(inputs["k_cache"]).astype(np.float16)
    v16 = np.asarray(inputs["v_cache"]).astype(np.float16)
    wa = np.asarray(inputs["w_attn"], np.float32).astype(np.float16)
    # wa_prep[nb, p, t, n] = w_attn[t*128+p, nb*512+n]
    wa_prep = np.ascontiguousarray(
        wa.reshape(8, 128, 6, 512).transpose(2, 1, 0, 3)
    )
    wp = np.asarray(inputs["w_proj"], np.float32).astype(np.float16)
    # wp_prep[p, h, n] = w_proj[h*64+p, n]
    wp_prep = np.ascontiguousarray(wp.reshape(H, 64, D).transpose(1, 0, 2))
    b_attn = np.ascontiguousarray(np.asarray(inputs["b_attn"], np.float32))
    b_proj = np.ascontiguousarray(np.asarray(inputs["b_proj"], np.float32))
    ident = np.eye(128, dtype=np.float16)
    in_maps = []
    for c in range(NCORES):
        sl = slice(c * BC, (c + 1) * BC)
        in_maps.append(
            {
                "hidden": np.ascontiguousarray(hidden16[sl]),
                "k_cache": np.ascontiguousarray(k16[sl]),
                "v_cache": np.ascontiguousarray(v16[sl]),
                "w_attn": wa_prep,
                "b_attn": b_attn,
                "w_proj": wp_prep,
                "b_proj": b_proj,
                "ident": ident,
            }
        )
    return in_maps


def kernel(**inputs):
    nc = _get_nc()
    in_maps = make_in_maps(inputs)
    res = run_bass_kernel_spmd(nc, in_maps, core_ids=list(range(NCORES)))
    out = np.empty((B, 1, D), np.float32)
    for c in range(NCORES):
        out[c * BC : (c + 1) * BC, 0, :] = np.asarray(res.results[c]["out"])
    return out


if __name__ == "__main__":
    build_kernel()
    print("kernel built OK")


# revision 34
# speedup vs baseline: 1.0864x; 1.0061x over previous
"""GPT2 paged-attention decode kernel for Trainium2 (Bass/Tile), 8-core SPMD.

Problem: B=32 batches, Q=1 query, D=1024, H=16 heads, DH=64, KV cache 8192.
  qkv = hidden @ w_attn + b_attn; split into q, k_cur, v_cur
  attention over concat(cache, current) per (b, h)  [no scaling, no mask]
  out = (softmax contexts) @ w_proj + b_proj

Sharding: pure batch data-parallel, 4 batches per core, zero collectives.

Key optimization vs the fp32 version: K/V caches (and weights/activations)
are cast to fp16 on the HOST, halving device HBM traffic (256MB -> 128MB per
core) and unlocking the fast 16-bit paths on every engine:
  - DVE tensor_tensor runs in 2x_1p mode (0.5 cyc/elem)
  - TensorE matmul streams 1 col/cycle (fp32 is 4 cyc/col)
Precision: products/partial sums in fp16 give |err(s)| ~ 5e-3 on scores of
magnitude ~20 -> ~0.5% weight error, far inside the 2e-2 gate.

Softmax max-subtraction is dropped (shift invariance; fp32/bf16 exp cannot
overflow for |s| <~ 30: e^30 ~ 1e13 << bf16/fp32 max). exp outputs are
stored bf16 (range to 3e38) for the fast matmul path; denominators are
accumulated in fp32 via the activation accum_out.

Per (b,h) pair on-chip schedule:
  - K/V fp16 tiles [128, 64, 64] DMA'd in natural row layout (partition p
    holds keys p*64..p*64+63 contiguously; 1MB per DMA).
  - VectorE: kq = K * q_broadcast (fp16 2x); 3-level binary-tree adds over
    the head dim (fp16 2x); final 8->1 reduce_sum to fp32 s [128, 64].
  - ScalarE: e = exp(s) -> bf16 with accum_out giving per-partition
    denominator partials (fp32).
  - TensorE: ctx row [1,64] accumulated in PSUM via 64 matmuls, e columns
    stationary (1-col LDWEIGHTS), V tiles streaming; denominators reduced
    across partitions with a ones-matmul into a shared [1,64] PSUM row;
    ctx rows transposed into columns of ctx^T via rank-1 matmuls.
  - The current token never enters the stream: it is folded in a small
    epilogue over all 64 pairs at once (PE transposes of the qkv rows,
    one elementwise mul + ones-matmul partition reduce + exp).
"""

import os
import sys

import numpy as np

sys.path.insert(0, "/opt/trn_rl_repo")

import concourse.bass as bass
import concourse.tile as tile
from concourse import bacc, mybir
from concourse.bass_utils import run_bass_kernel_spmd

FP32 = mybir.dt.float32
FP16 = mybir.dt.float16
BF16 = mybir.dt.bfloat16

# Problem shape (hardcoded per contest rules).
B, D, H, DH, KV = 32, 1024, 16, 64, 8192
NCORES = 8
BC = B // NCORES          # batches per core = 4
NPAIRS = BC * H           # 64 (b,h) pairs per core
JT = KV // 128            # 64 key-slots per partition


def _bcast_ap(t_ap, col0, ncols, nparts, row_stride):
    """DRAM AP [nparts, rows, ncols] with partition stride 0 (broadcast)."""
    return bass.AP(
        tensor=t_ap.tensor,
        offset=t_ap.offset + col0,
        ap=[[0, nparts], [row_stride, t_ap.shape[0]], [1, ncols]],
    )


def build_kernel():
    nc = bacc.Bacc(
        "TRN2",
        target_bir_lowering=False,
        debug=False,
        enable_asserts=False,
        num_devices=NCORES,
    )

    hidden = nc.dram_tensor("hidden", [BC, D], FP16, kind="ExternalInput")
    k_cache = nc.dram_tensor("k_cache", [BC, H, KV, DH], FP16, kind="ExternalInput")
    v_cache = nc.dram_tensor("v_cache", [BC, H, KV, DH], FP16, kind="ExternalInput")
    # host-prepped: wa_prep[nb, p, t, n] = w_attn[t*128+p, nb*512+n]  (fp16)
    w_attn = nc.dram_tensor("w_attn", [6, 128, 8, 512], FP16, kind="ExternalInput")
    b_attn = nc.dram_tensor("b_attn", [3 * D], FP32, kind="ExternalInput")
    # host-prepped: wp_prep[p, h, n] = w_proj[h*64+p, n]  (fp16)
    w_proj = nc.dram_tensor("w_proj", [64, H, D], FP16, kind="ExternalInput")
    b_proj = nc.dram_tensor("b_proj", [D], FP32, kind="ExternalInput")
    ident = nc.dram_tensor("ident", [128, 128], FP16, kind="ExternalInput")
    selector = nc.dram_tensor("selector", [BC, BC, 128], FP16, kind="ExternalInput")
    out = nc.dram_tensor("out", [BC, D], FP32, kind="ExternalOutput")

    with tile.TileContext(nc) as tc:
        with (
            tc.tile_pool(name="singles", bufs=1) as singles,
            tc.tile_pool(name="wa_pool", bufs=2) as wa_pool,
            tc.tile_pool(name="kv_pool", bufs=3) as kv_pool,
            tc.tile_pool(name="se_pool", bufs=4) as se_pool,
            tc.tile_pool(name="mm512", bufs=2, space="PSUM") as mm512,
            tc.tile_pool(name="ctx_ps", bufs=2, space="PSUM") as ctx_ps,
            tc.tile_pool(name="ctxt_ps", bufs=1, space="PSUM") as ctxt_ps,
            tc.tile_pool(name="l_ps", bufs=1, space="PSUM") as l_ps,
            tc.tile_pool(name="po_ps", bufs=2, space="PSUM") as po_ps,
        ):
            # ---- constants ----
            ones32 = singles.tile([128, 64], FP32)
            nc.vector.memset(ones32, 1.0)
            ones16 = singles.tile([128, 64], FP16)
            nc.vector.memset(ones16, 1.0)
            id_sb = singles.tile([128, 128], FP16)
            nc.gpsimd.dma_start(out=id_sb, in_=ident.ap())

            ba_sb = singles.tile([BC, 3 * D], FP32)
            nc.gpsimd.dma_start(
                out=ba_sb, in_=_bcast_ap(b_attn.ap().unsqueeze(0), 0, 3 * D, BC, 0)
            )
            bp_sb = singles.tile([BC, D], FP32)
            nc.gpsimd.dma_start(
                out=bp_sb, in_=_bcast_ap(b_proj.ap().unsqueeze(0), 0, D, BC, 0)
            )
            # batch-selector rows: sel[:, b, :] is [4, 128] with row b all-ones
            # (used as matmul lhsT to broadcast q row b across 128 partitions)
            sel = singles.tile([BC, BC, 128], FP16)
            nc.gpsimd.dma_start(out=sel, in_=selector.ap())

            # ---- hidden -> hT via PE transposes ----
            hid_sb = singles.tile([BC, D], FP16)
            nc.gpsimd.dma_start(out=hid_sb, in_=hidden.ap())
            ps_hT = mm512.tile([128, 8, BC], FP16, tag="mm512", name="ps_hT")
            for t in range(8):
                nc.tensor.transpose(
                    ps_hT[:, t, :],
                    hid_sb[:, t * 128 : (t + 1) * 128],
                    id_sb[0:BC, 0:BC],
                )
            hT = singles.tile([128, 8, BC], FP16)
            nc.scalar.copy(hT, ps_hT)

            # ---- QKV projection: qkv = hidden @ w_attn + b_attn ----
            # q third first (unblocks the attention stream), then k/v thirds.
            qkv_sb = singles.tile([BC, 3 * D], FP16)
            qb_all = singles.tile([128, BC, D], FP16)

            def qkv_block(nb):
                wa_t = wa_pool.tile([128, 8, 512], FP16, tag="wa")
                nc.gpsimd.dma_start(out=wa_t, in_=w_attn.ap()[nb])
                ps = mm512.tile([BC, 512], FP32, tag="mm512")
                for t in range(8):
                    nc.tensor.matmul(
                        ps,
                        hT[:, t, :],
                        wa_t[:, t, :],
                        start=(t == 0),
                        stop=(t == 7),
                    )
                nc.vector.tensor_add(
                    qkv_sb[:, nb * 512 : (nb + 1) * 512],
                    ps,
                    ba_sb[:, nb * 512 : (nb + 1) * 512],
                )
                if nb in (0, 1):
                    # qb_all[p, b, :] = q row of batch b on every partition:
                    # rank-K=4 matmuls with one-hot-row selectors (stays off
                    # the DMA queues entirely)
                    for b in range(BC):
                        ps_qb = mm512.tile(
                            [128, 512], FP32, tag="mm512", name=f"ps_qb{nb}_{b}"
                        )
                        nc.tensor.matmul(
                            ps_qb,
                            sel[:, b, :],
                            qkv_sb[:, nb * 512 : (nb + 1) * 512],
                            start=True,
                            stop=True,
                        )
                        nc.scalar.copy(
                            qb_all[:, b, nb * 512 : (nb + 1) * 512], ps_qb
                        )

            # only the q third now — the k_cur/v_cur thirds are emitted
            # after the stream has started (they'd head-of-line-block the
            # stream's DVE/PE work otherwise)
            qkv_block(0)
            qkv_block(1)

            # ---- w_proj (after the wa chunks in the gpsimd DMA FIFO; needed
            # from the first per-head-pair c_proj block onward) ----
            wp_sb = singles.tile([64, H, D], FP16)
            nc.gpsimd.dma_start(out=wp_sb, in_=w_proj.ap())

            # current-token epilogue tiles (filled by cur_token_block below,
            # emitted once the stream is in flight)
            e_cur = singles.tile([1, NPAIRS], FP32)
            vc = singles.tile([64, NPAIRS], FP32)

            def cur_token_block():
                # Transpose q/k_cur/v_cur rows [4, 64] -> [64, (h, b)] tiles.
                ps_tr = mm512.tile([64, 3, H, BC], FP16, tag="mm512", name="ps_tr")
                for part in range(3):
                    for h in range(H):
                        nc.tensor.transpose(
                            ps_tr[:, part, h, :],
                            qkv_sb[:, part * D + h * DH : part * D + (h + 1) * DH],
                            id_sb[0:BC, 0:BC],
                        )
                qT = singles.tile([64, H, BC], FP16)
                nc.scalar.copy(qT, ps_tr[:, 0])
                kT = singles.tile([64, H, BC], FP16)
                nc.scalar.copy(kT, ps_tr[:, 1])
                vT = singles.tile([64, H, BC], FP32)
                nc.vector.tensor_copy(vT, ps_tr[:, 2])

                # s_cur[j] = sum_d q[d, j]*k_cur[d, j] (partition reduce by MM)
                qk = singles.tile([64, H, BC], FP16)
                nc.vector.tensor_mul(qk, qT, kT)
                ps_scur = mm512.tile([1, NPAIRS], FP32, tag="mm512", name="ps_scur")
                nc.tensor.matmul(
                    ps_scur, ones16[0:64, 0:1], qk, start=True, stop=True
                )
                nc.scalar.activation(
                    e_cur, ps_scur, mybir.ActivationFunctionType.Exp
                )
                # broadcast e_cur across 64 partitions (ones outer product)
                ps_eb = mm512.tile([64, NPAIRS], FP32, tag="mm512", name="ps_eb")
                nc.tensor.matmul(
                    ps_eb, ones32[0:1, 0:64], e_cur, start=True, stop=True
                )
                eb_sb = singles.tile([64, NPAIRS], FP32)
                nc.scalar.copy(eb_sb, ps_eb)
                nc.vector.tensor_mul(vc, vT, eb_sb)

            # ---- attention stream over pairs (h-major: j = h*BC + b) ----
            psum_ctxT = ctxt_ps.tile([64, NPAIRS], FP32)  # ctx^T, one col per pair
            psum_l = l_ps.tile([1, NPAIRS], FP32)         # softmax denominators
            # c_proj accumulators live across the whole stream (filled
            # incrementally as each head pair completes)
            ps_o = [
                po_ps.tile([BC, 512], FP32, tag="po", name=f"ps_o{i}")
                for i in range(2)
            ]

            # Two (b, h) pairs per group (adjacent heads of one batch): DVE
            # ops process both pairs' tiles at once to amortize per-op cost.
            for g in range(NPAIRS // 2):
                hh, b = g // BC, g % BC
                h0 = 2 * hh

                if g == 2:
                    # stream is rolling: now emit the k_cur/v_cur qkv thirds
                    # and the current-token preparation
                    for nb in (2, 3, 4, 5):
                        qkv_block(nb)
                    cur_token_block()

                k2 = kv_pool.tile([128, 2, JT, DH], FP16, tag="k")
                v2 = kv_pool.tile([128, 2, JT, DH], FP16, tag="v")
                for i in range(2):
                    nc.sync.dma_start(
                        out=k2[:, i],
                        in_=k_cache.ap()[b, h0 + i].rearrange(
                            "(p jj) d -> p jj d", jj=JT
                        ),
                    )
                    nc.scalar.dma_start(
                        out=v2[:, i],
                        in_=v_cache.ap()[b, h0 + i].rearrange(
                            "(p jj) d -> p jj d", jj=JT
                        ),
                    )

                # s[p, hh, jj] = sum_d K[p, hh, jj, d] * q_hh[d]
                # fp16 2x mul + binary-tree halving adds, all in-place in k2
                # (K is dead after the mul; saves a whole SBUF pool)
                qb = (
                    qb_all[:, b, h0 * DH : (h0 + 2) * DH]
                    .rearrange("p (i d) -> p i d", i=2)
                    .unsqueeze(2)
                    .broadcast_to([128, 2, JT, DH])
                )
                nc.vector.tensor_mul(k2, k2, qb)
                nc.vector.tensor_add(
                    k2[:, :, :, 0:32], k2[:, :, :, 0:32], k2[:, :, :, 32:64]
                )
                nc.vector.tensor_add(
                    k2[:, :, :, 0:16], k2[:, :, :, 0:16], k2[:, :, :, 16:32]
                )
                nc.vector.tensor_add(
                    k2[:, :, :, 0:8], k2[:, :, :, 0:8], k2[:, :, :, 8:16]
                )
                s2 = se_pool.tile([128, 2, JT], FP32, tag="s")
                nc.vector.reduce_sum(s2, k2[:, :, :, 0:8], axis=mybir.AxisListType.X)

                # per pair: exp (bf16) + denominator partials, ctx matmuls
                e2 = se_pool.tile([128, 2, JT], BF16, tag="e")
                for i in range(2):
                    j = (h0 + i) * BC + b
                    lp = se_pool.tile([128, 1], FP32, tag="lp")
                    nc.scalar.activation(
                        e2[:, i],
                        s2[:, i],
                        mybir.ActivationFunctionType.Exp,
                        accum_out=lp,
                    )
                    psum_row = ctx_ps.tile([1, DH], FP32, tag="ctx")
                    for jj in range(JT):
                        nc.tensor.matmul(
                            psum_row,
                            e2[:, i, jj : jj + 1],
                            v2[:, i, jj, :],
                            start=(jj == 0),
                            stop=(jj == JT - 1),
                        )
                    # denominator: l[j] = sum_p lp[p]
                    nc.tensor.matmul(
                        psum_l[:, j : j + 1],
                        lp,
                        ones32[:, 0:1],
                        start=True,
                        stop=True,
                        skip_group_check=True,
                    )
                    # transpose the row into column j of ctx^T (rank-1 matmul)
                    ctx_row = se_pool.tile([1, DH], FP32, tag="ctxrow")
                    nc.scalar.copy(ctx_row, psum_row)
                    nc.tensor.matmul(
                        psum_ctxT[:, j : j + 1],
                        ctx_row,
                        ones32[0:1, 0:1],
                        start=True,
                        stop=True,
                        skip_group_check=True,
                    )

                if b == BC - 1:
                    # Head pair hh complete for all batches: fold in the
                    # current token, normalize, and run its c_proj matmuls
                    # now (keeps the post-stream tail tiny).
                    sl = slice(h0 * BC, (h0 + 2) * BC)
                    lt = se_pool.tile([1, 2 * BC], FP32, tag="lt")
                    nc.vector.tensor_add(lt, psum_l[:, sl], e_cur[:, sl])
                    rr = se_pool.tile([1, 2 * BC], FP32, tag="rr")
                    nc.vector.reciprocal(rr, lt)
                    ps_rb = ctx_ps.tile(
                        [64, 2 * BC], FP32, tag="ctx", name=f"ps_rb{hh}"
                    )
                    nc.tensor.matmul(
                        ps_rb,
                        ones32[0:1, 0:64],
                        rr,
                        start=True,
                        stop=True,
                        skip_group_check=True,
                    )
                    rb8 = se_pool.tile([64, 2 * BC], FP32, tag="rb8")
                    nc.scalar.copy(rb8, ps_rb)
                    cf8 = se_pool.tile([64, 2 * BC], FP32, tag="cf8")
                    nc.vector.tensor_add(cf8, psum_ctxT[:, sl], vc[:, sl])
                    cs8 = se_pool.tile([64, 2 * BC], FP16, tag="cs8")
                    nc.vector.tensor_mul(cs8, cf8, rb8)
                    for i in range(2):
                        for nb in range(2):
                            nc.tensor.matmul(
                                ps_o[nb],
                                cs8[:, i * BC : (i + 1) * BC],
                                wp_sb[:, h0 + i, nb * 512 : (nb + 1) * 512],
                                start=(h0 + i == 0),
                                stop=(h0 + i == H - 1),
                                skip_group_check=True,
                            )

            # ---- tail: output bias + store ----
            out_sb = singles.tile([BC, D], FP32)
            for nb in range(2):
                nc.vector.tensor_add(
                    out_sb[:, nb * 512 : (nb + 1) * 512],
                    ps_o[nb],
                    bp_sb[:, nb * 512 : (nb + 1) * 512],
                )
            nc.gpsimd.dma_start(out=out.ap(), in_=out_sb)

    nc.compile()
    return nc


_NC_CACHE = None


def _get_nc():
    global _NC_CACHE
    if _NC_CACHE is None:
        _NC_CACHE = build_kernel()
    return _NC_CACHE


def make_in_maps(inputs):
    """Shard full inputs into per-core input maps (batch data-parallel).

    Heavy tensors are cast to fp16 host-side: the device kernel streams
    fp16, halving HBM traffic.
    """
    hidden = np.asarray(inputs["hidden_states"], np.float32).reshape(B, D)
    hidden16 = np.ascontiguousarray(hidden.astype(np.float16))
    k16 = np.asarray(inputs["k_cache"]).astype(np.float16)
    v16 = np.asarray(inputs["v_cache"]).astype(np.float16)
    wa = np.asarray(inputs["w_attn"], np.float32).astype(np.float16)
    # wa_prep[nb, p, t, n] = w_attn[t*128+p, nb*512+n]
    wa_prep = np.ascontiguousarray(
        wa.reshape(8, 128, 6, 512).transpose(2, 1, 0, 3)
    )
    wp = np.asarray(inputs["w_proj"], np.float32).astype(np.float16)
    # wp_prep[p, h, n] = w_proj[h*64+p, n]
    wp_prep = np.ascontiguousarray(wp.reshape(H, 64, D).transpose(1, 0, 2))
    b_attn = np.ascontiguousarray(np.asarray(inputs["b_attn"], np.float32))
    b_proj = np.ascontiguousarray(np.asarray(inputs["b_proj"], np.float32))
    ident = np.eye(128, dtype=np.float16)
    selector = np.zeros((BC, BC, 128), np.float16)
    for b in range(BC):
        selector[b, b, :] = 1.0
    in_maps = []
    for c in range(NCORES):
        sl = slice(c * BC, (c + 1) * BC)
        in_maps.append(
            {
                "hidden": np.ascontiguousarray(hidden16[sl]),
                "k_cache": np.ascontiguousarray(k16[sl]),
                "v_cache": np.ascontiguousarray(v16[sl]),
                "w_attn": wa_prep,
                "b_attn": b_attn,
                "w_proj": wp_prep,
                "b_proj": b_proj,
                "ident": ident,
                "selector": selector,
            }
        )
    return in_maps


def kernel(**inputs):
    nc = _get_nc()
    in_maps = make_in_maps(inputs)
    res = run_bass_kernel_spmd(nc, in_maps, core_ids=list(range(NCORES)))
    out = np.empty((B, 1, D), np.float32)
    for c in range(NCORES):
        out[c * BC : (c + 1) * BC, 0, :] = np.asarray(res.results[c]["out"])
    return out


if __name__ == "__main__":
    build_kernel()
    print("kernel built OK")


# revision 38
# speedup vs baseline: 1.0901x; 1.0035x over previous
"""GPT2 paged-attention decode kernel for Trainium2 (Bass/Tile), 8-core SPMD.

Problem: B=32 batches, Q=1 query, D=1024, H=16 heads, DH=64, KV cache 8192.
  qkv = hidden @ w_attn + b_attn; split into q, k_cur, v_cur
  attention over concat(cache, current) per (b, h)  [no scaling, no mask]
  out = (softmax contexts) @ w_proj + b_proj

Sharding: pure batch data-parallel, 4 batches per core, zero collectives.

Key optimization vs the fp32 version: K/V caches (and weights/activations)
are cast to fp16 on the HOST, halving device HBM traffic (256MB -> 128MB per
core) and unlocking the fast 16-bit paths on every engine:
  - DVE tensor_tensor runs in 2x_1p mode (0.5 cyc/elem)
  - TensorE matmul streams 1 col/cycle (fp32 is 4 cyc/col)
Precision: products/partial sums in fp16 give |err(s)| ~ 5e-3 on scores of
magnitude ~20 -> ~0.5% weight error, far inside the 2e-2 gate.

Softmax max-subtraction is dropped (shift invariance; fp32/bf16 exp cannot
overflow for |s| <~ 30: e^30 ~ 1e13 << bf16/fp32 max). exp outputs are
stored bf16 (range to 3e38) for the fast matmul path; denominators are
accumulated in fp32 via the activation accum_out.

Per (b,h) pair on-chip schedule:
  - K/V fp16 tiles [128, 64, 64] DMA'd in natural row layout (partition p
    holds keys p*64..p*64+63 contiguously; 1MB per DMA).
  - VectorE: kq = K * q_broadcast (fp16 2x); 3-level binary-tree adds over
    the head dim (fp16 2x); final 8->1 reduce_sum to fp32 s [128, 64].
  - ScalarE: e = exp(s) -> bf16 with accum_out giving per-partition
    denominator partials (fp32).
  - TensorE: ctx row [1,64] accumulated in PSUM via 64 matmuls, e columns
    stationary (1-col LDWEIGHTS), V tiles streaming; denominators reduced
    across partitions with a ones-matmul into a shared [1,64] PSUM row;
    ctx rows transposed into columns of ctx^T via rank-1 matmuls.
  - The current token never enters the stream: it is folded in a small
    epilogue over all 64 pairs at once (PE transposes of the qkv rows,
    one elementwise mul + ones-matmul partition reduce + exp).
"""

import os
import sys

import numpy as np

sys.path.insert(0, "/opt/trn_rl_repo")

import concourse.bass as bass
import concourse.tile as tile
from concourse import bacc, mybir
from concourse.bass_utils import run_bass_kernel_spmd

FP32 = mybir.dt.float32
FP16 = mybir.dt.float16
BF16 = mybir.dt.bfloat16

# Problem shape (hardcoded per contest rules).
B, D, H, DH, KV = 32, 1024, 16, 64, 8192
NCORES = 8
BC = B // NCORES          # batches per core = 4
NPAIRS = BC * H           # 64 (b,h) pairs per core
JT = KV // 128            # 64 key-slots per partition


def _bcast_ap(t_ap, col0, ncols, nparts, row_stride):
    """DRAM AP [nparts, rows, ncols] with partition stride 0 (broadcast)."""
    return bass.AP(
        tensor=t_ap.tensor,
        offset=t_ap.offset + col0,
        ap=[[0, nparts], [row_stride, t_ap.shape[0]], [1, ncols]],
    )


def build_kernel():
    nc = bacc.Bacc(
        "TRN2",
        target_bir_lowering=False,
        debug=False,
        enable_asserts=False,
        num_devices=NCORES,
    )

    hidden = nc.dram_tensor("hidden", [BC, D], FP16, kind="ExternalInput")
    k_cache = nc.dram_tensor("k_cache", [BC, H, KV, DH], FP16, kind="ExternalInput")
    v_cache = nc.dram_tensor("v_cache", [BC, H, KV, DH], FP16, kind="ExternalInput")
    # host-prepped: wa_prep[nb, p, t, n] = w_attn[t*128+p, nb*512+n]  (fp16)
    w_attn = nc.dram_tensor("w_attn", [6, 128, 8, 512], FP16, kind="ExternalInput")
    b_attn = nc.dram_tensor("b_attn", [3 * D], FP32, kind="ExternalInput")
    # host-prepped: wp_prep[p, h, n] = w_proj[h*64+p, n]  (fp16)
    w_proj = nc.dram_tensor("w_proj", [64, H, D], FP16, kind="ExternalInput")
    b_proj = nc.dram_tensor("b_proj", [D], FP32, kind="ExternalInput")
    ident = nc.dram_tensor("ident", [128, 128], FP16, kind="ExternalInput")
    selector = nc.dram_tensor("selector", [BC, BC, 128], FP16, kind="ExternalInput")
    out = nc.dram_tensor("out", [BC, D], FP32, kind="ExternalOutput")

    with tile.TileContext(nc) as tc:
        with (
            tc.tile_pool(name="singles", bufs=1) as singles,
            tc.tile_pool(name="wa_pool", bufs=2) as wa_pool,
            tc.tile_pool(name="kv_pool", bufs=3) as kv_pool,
            tc.tile_pool(name="se_pool", bufs=4) as se_pool,
            tc.tile_pool(name="mm512", bufs=2, space="PSUM") as mm512,
            tc.tile_pool(name="ctx_ps", bufs=2, space="PSUM") as ctx_ps,
            tc.tile_pool(name="ctxt_ps", bufs=2, space="PSUM") as ctxt_ps,
            tc.tile_pool(name="po_ps", bufs=2, space="PSUM") as po_ps,
        ):
            # ---- constants ----
            ones32 = singles.tile([128, 64], FP32)
            nc.vector.memset(ones32, 1.0)
            ones16 = singles.tile([128, 64], FP16)
            nc.vector.memset(ones16, 1.0)
            id_sb = singles.tile([128, 128], FP16)
            nc.gpsimd.dma_start(out=id_sb, in_=ident.ap())

            ba_sb = singles.tile([BC, 3 * D], FP32)
            nc.gpsimd.dma_start(
                out=ba_sb, in_=_bcast_ap(b_attn.ap().unsqueeze(0), 0, 3 * D, BC, 0)
            )
            bp_sb = singles.tile([BC, D], FP32)
            nc.gpsimd.dma_start(
                out=bp_sb, in_=_bcast_ap(b_proj.ap().unsqueeze(0), 0, D, BC, 0)
            )
            # batch-selector rows: sel[:, b, :] is [4, 128] with row b all-ones
            # (used as matmul lhsT to broadcast q row b across 128 partitions)
            sel = singles.tile([BC, BC, 128], FP16)
            nc.gpsimd.dma_start(out=sel, in_=selector.ap())

            # ---- hidden -> hT via PE transposes ----
            hid_sb = singles.tile([BC, D], FP16)
            nc.gpsimd.dma_start(out=hid_sb, in_=hidden.ap())
            ps_hT = mm512.tile([128, 8, BC], FP16, tag="mm512", name="ps_hT")
            for t in range(8):
                nc.tensor.transpose(
                    ps_hT[:, t, :],
                    hid_sb[:, t * 128 : (t + 1) * 128],
                    id_sb[0:BC, 0:BC],
                )
            hT = singles.tile([128, 8, BC], FP16)
            nc.scalar.copy(hT, ps_hT)

            # ---- QKV projection: qkv = hidden @ w_attn + b_attn ----
            # q third first (unblocks the attention stream), then k/v thirds.
            qkv_sb = singles.tile([BC, 3 * D], FP16)
            qb_all = singles.tile([128, BC, D], FP16)

            def qkv_block(nb):
                wa_t = wa_pool.tile([128, 8, 512], FP16, tag="wa")
                nc.gpsimd.dma_start(out=wa_t, in_=w_attn.ap()[nb])
                ps = mm512.tile([BC, 512], FP32, tag="mm512")
                for t in range(8):
                    nc.tensor.matmul(
                        ps,
                        hT[:, t, :],
                        wa_t[:, t, :],
                        start=(t == 0),
                        stop=(t == 7),
                    )
                nc.vector.tensor_add(
                    qkv_sb[:, nb * 512 : (nb + 1) * 512],
                    ps,
                    ba_sb[:, nb * 512 : (nb + 1) * 512],
                )
                if nb in (0, 1):
                    # qb_all[p, b, :] = q row of batch b on every partition:
                    # rank-K=4 matmuls with one-hot-row selectors (stays off
                    # the DMA queues entirely)
                    for b in range(BC):
                        ps_qb = mm512.tile(
                            [128, 512], FP32, tag="mm512", name=f"ps_qb{nb}_{b}"
                        )
                        nc.tensor.matmul(
                            ps_qb,
                            sel[:, b, :],
                            qkv_sb[:, nb * 512 : (nb + 1) * 512],
                            start=True,
                            stop=True,
                        )
                        nc.scalar.copy(
                            qb_all[:, b, nb * 512 : (nb + 1) * 512], ps_qb
                        )

            # only the q third now — the k_cur/v_cur thirds are emitted
            # after the stream has started (they'd head-of-line-block the
            # stream's DVE/PE work otherwise)
            qkv_block(0)
            qkv_block(1)

            # ---- w_proj (after the wa chunks in the gpsimd DMA FIFO; needed
            # from the first per-head-pair c_proj block onward) ----
            wp_sb = singles.tile([64, H, D], FP16)
            nc.gpsimd.dma_start(out=wp_sb, in_=w_proj.ap())

            # current-token epilogue tiles (filled by cur_token_block below,
            # emitted once the stream is in flight)
            e_cur = singles.tile([1, NPAIRS], FP32)
            vc = singles.tile([64, NPAIRS], FP32)

            def cur_token_block():
                # Transpose q/k_cur/v_cur rows [4, 64] -> [64, (h, b)] tiles.
                ps_tr = mm512.tile([64, 3, H, BC], FP16, tag="mm512", name="ps_tr")
                for part in range(3):
                    for h in range(H):
                        nc.tensor.transpose(
                            ps_tr[:, part, h, :],
                            qkv_sb[:, part * D + h * DH : part * D + (h + 1) * DH],
                            id_sb[0:BC, 0:BC],
                        )
                qT = singles.tile([64, H, BC], FP16)
                nc.scalar.copy(qT, ps_tr[:, 0])
                kT = singles.tile([64, H, BC], FP16)
                nc.scalar.copy(kT, ps_tr[:, 1])
                vT = singles.tile([64, H, BC], FP32)
                nc.vector.tensor_copy(vT, ps_tr[:, 2])

                # s_cur[j] = sum_d q[d, j]*k_cur[d, j] (partition reduce by MM)
                qk = singles.tile([64, H, BC], FP16)
                nc.vector.tensor_mul(qk, qT, kT)
                ps_scur = mm512.tile([1, NPAIRS], FP32, tag="mm512", name="ps_scur")
                nc.tensor.matmul(
                    ps_scur, ones16[0:64, 0:1], qk, start=True, stop=True
                )
                nc.scalar.activation(
                    e_cur, ps_scur, mybir.ActivationFunctionType.Exp
                )
                # broadcast e_cur across 64 partitions (ones outer product)
                ps_eb = mm512.tile([64, NPAIRS], FP32, tag="mm512", name="ps_eb")
                nc.tensor.matmul(
                    ps_eb, ones32[0:1, 0:64], e_cur, start=True, stop=True
                )
                eb_sb = singles.tile([64, NPAIRS], FP32)
                nc.scalar.copy(eb_sb, ps_eb)
                nc.vector.tensor_mul(vc, vT, eb_sb)

            # ---- attention stream over pairs (h-major: j = h*BC + b) ----
            # c_proj accumulators live across the whole stream (filled
            # incrementally as each head pair completes)
            ps_o = [
                po_ps.tile([BC, 512], FP32, tag="po", name=f"ps_o{i}")
                for i in range(2)
            ]
            ctxT_hh = None  # per-head-pair [65, 8] tile: rows 0-63 = ctx^T
            # columns for (i, b); row 64 = softmax denominators

            # Two (b, h) pairs per group (adjacent heads of one batch): DVE
            # ops process both pairs' tiles at once to amortize per-op cost.
            for g in range(NPAIRS // 2):
                hh, b = g // BC, g % BC
                h0 = 2 * hh

                if g == 2:
                    # stream is rolling: now emit the k_cur/v_cur qkv thirds
                    # and the current-token preparation
                    for nb in (2, 3, 4, 5):
                        qkv_block(nb)
                    cur_token_block()

                if b == 0:
                    ctxT_hh = ctxt_ps.tile(
                        [65, 2 * BC], FP32, tag="ctxt", name=f"ctxT{hh}"
                    )

                k2 = kv_pool.tile([128, 2, JT, DH], FP16, tag="k")
                v2 = kv_pool.tile([128, 2, JT, DH], FP16, tag="v")
                for i in range(2):
                    nc.sync.dma_start(
                        out=k2[:, i],
                        in_=k_cache.ap()[b, h0 + i].rearrange(
                            "(p jj) d -> p jj d", jj=JT
                        ),
                    )
                    nc.scalar.dma_start(
                        out=v2[:, i],
                        in_=v_cache.ap()[b, h0 + i].rearrange(
                            "(p jj) d -> p jj d", jj=JT
                        ),
                    )

                # s[p, hh, jj] = sum_d K[p, hh, jj, d] * q_hh[d]
                # fp16 2x mul + binary-tree halving adds, all in-place in k2
                # (K is dead after the mul; saves a whole SBUF pool)
                qb = (
                    qb_all[:, b, h0 * DH : (h0 + 2) * DH]
                    .rearrange("p (i d) -> p i d", i=2)
                    .unsqueeze(2)
                    .broadcast_to([128, 2, JT, DH])
                )
                nc.vector.tensor_mul(k2, k2, qb)
                nc.vector.tensor_add(
                    k2[:, :, :, 0:32], k2[:, :, :, 0:32], k2[:, :, :, 32:64]
                )
                nc.vector.tensor_add(
                    k2[:, :, :, 0:16], k2[:, :, :, 0:16], k2[:, :, :, 16:32]
                )
                nc.vector.tensor_add(
                    k2[:, :, :, 0:8], k2[:, :, :, 0:8], k2[:, :, :, 8:16]
                )
                s2 = se_pool.tile([128, 2, JT], FP32, tag="s")
                nc.vector.reduce_sum(s2, k2[:, :, :, 0:8], axis=mybir.AxisListType.X)

                # per pair: exp (bf16) + denominator partials, ctx matmuls
                e2 = se_pool.tile([128, 2, JT], BF16, tag="e")
                for i in range(2):
                    col = i * BC + b
                    lp = se_pool.tile([128, 1], FP32, tag="lp")
                    nc.scalar.activation(
                        e2[:, i],
                        s2[:, i],
                        mybir.ActivationFunctionType.Exp,
                        accum_out=lp,
                    )
                    psum_row = ctx_ps.tile([1, DH], FP32, tag="ctx")
                    for jj in range(JT):
                        nc.tensor.matmul(
                            psum_row,
                            e2[:, i, jj : jj + 1],
                            v2[:, i, jj, :],
                            start=(jj == 0),
                            stop=(jj == JT - 1),
                        )
                    # denominator into row 64 of the head-pair tile
                    nc.tensor.matmul(
                        ctxT_hh[64:65, col : col + 1],
                        lp,
                        ones32[:, 0:1],
                        start=True,
                        stop=True,
                        skip_group_check=True,
                    )
                    # transpose the row into the (i, b) column of ctx^T
                    ctx_row = se_pool.tile([1, DH], FP32, tag="ctxrow")
                    nc.scalar.copy(ctx_row, psum_row)
                    nc.tensor.matmul(
                        ctxT_hh[0:64, col : col + 1],
                        ctx_row,
                        ones32[0:1, 0:1],
                        start=True,
                        stop=True,
                        skip_group_check=True,
                    )

                if b == BC - 1:
                    # Head pair hh complete for all batches: fold in the
                    # current token, normalize, and run its c_proj matmuls
                    # now (keeps the post-stream tail tiny).
                    sl = slice(h0 * BC, (h0 + 2) * BC)
                    lt = se_pool.tile([1, 2 * BC], FP32, tag="lt")
                    nc.vector.tensor_add(lt, ctxT_hh[64:65, :], e_cur[:, sl])
                    rr = se_pool.tile([1, 2 * BC], FP32, tag="rr")
                    nc.vector.reciprocal(rr, lt)
                    ps_rb = ctx_ps.tile(
                        [64, 2 * BC], FP32, tag="ctx", name=f"ps_rb{hh}"
                    )
                    nc.tensor.matmul(
                        ps_rb,
                        ones32[0:1, 0:64],
                        rr,
                        start=True,
                        stop=True,
                        skip_group_check=True,
                    )
                    rb8 = se_pool.tile([64, 2 * BC], FP32, tag="rb8")
                    nc.scalar.copy(rb8, ps_rb)
                    cf8 = se_pool.tile([64, 2 * BC], FP32, tag="cf8")
                    nc.vector.tensor_add(cf8, ctxT_hh[0:64, :], vc[:, sl])
                    cs8 = se_pool.tile([64, 2 * BC], FP16, tag="cs8")
                    nc.vector.tensor_mul(cs8, cf8, rb8)
                    for i in range(2):
                        for nb in range(2):
                            nc.tensor.matmul(
                                ps_o[nb],
                                cs8[:, i * BC : (i + 1) * BC],
                                wp_sb[:, h0 + i, nb * 512 : (nb + 1) * 512],
                                start=(h0 + i == 0),
                                stop=(h0 + i == H - 1),
                                skip_group_check=True,
                            )

            # ---- tail: output bias + store ----
            out_sb = singles.tile([BC, D], FP32)
            for nb in range(2):
                nc.vector.tensor_add(
                    out_sb[:, nb * 512 : (nb + 1) * 512],
                    ps_o[nb],
                    bp_sb[:, nb * 512 : (nb + 1) * 512],
                )
            nc.gpsimd.dma_start(out=out.ap(), in_=out_sb)

    nc.compile()
    return nc


_NC_CACHE = None


def _get_nc():
    global _NC_CACHE
    if _NC_CACHE is None:
        _NC_CACHE = build_kernel()
    return _NC_CACHE


def make_in_maps(inputs):
    """Shard full inputs into per-core input maps (batch data-parallel).

    Heavy tensors are cast to fp16 host-side: the device kernel streams
    fp16, halving HBM traffic.
    """
    hidden = np.asarray(inputs["hidden_states"], np.float32).reshape(B, D)
    hidden16 = np.ascontiguousarray(hidden.astype(np.float16))
    k16 = np.asarray(inputs["k_cache"]).astype(np.float16)
    v16 = np.asarray(inputs["v_cache"]).astype(np.float16)
    wa = np.asarray(inputs["w_attn"], np.float32).astype(np.float16)
    # wa_prep[nb, p, t, n] = w_attn[t*128+p, nb*512+n]
    wa_prep = np.ascontiguousarray(
        wa.reshape(8, 128, 6, 512).transpose(2, 1, 0, 3)
    )
    wp = np.asarray(inputs["w_proj"], np.float32).astype(np.float16)
    # wp_prep[p, h, n] = w_proj[h*64+p, n]
    wp_prep = np.ascontiguousarray(wp.reshape(H, 64, D).transpose(1, 0, 2))
    b_attn = np.ascontiguousarray(np.asarray(inputs["b_attn"], np.float32))
    b_proj = np.ascontiguousarray(np.asarray(inputs["b_proj"], np.float32))
    ident = np.eye(128, dtype=np.float16)
    selector = np.zeros((BC, BC, 128), np.float16)
    for b in range(BC):
        selector[b, b, :] = 1.0
    in_maps = []
    for c in range(NCORES):
        sl = slice(c * BC, (c + 1) * BC)
        in_maps.append(
            {
                "hidden": np.ascontiguousarray(hidden16[sl]),
                "k_cache": np.ascontiguousarray(k16[sl]),
                "v_cache": np.ascontiguousarray(v16[sl]),
                "w_attn": wa_prep,
                "b_attn": b_attn,
                "w_proj": wp_prep,
                "b_proj": b_proj,
                "ident": ident,
                "selector": selector,
            }
        )
    return in_maps


def kernel(**inputs):
    nc = _get_nc()
    in_maps = make_in_maps(inputs)
    res = run_bass_kernel_spmd(nc, in_maps, core_ids=list(range(NCORES)))
    out = np.empty((B, 1, D), np.float32)
    for c in range(NCORES):
        out[c * BC : (c + 1) * BC, 0, :] = np.asarray(res.results[c]["out"])
    return out


if __name__ == "__main__":
    build_kernel()
    print("kernel built OK")


# revision 42
# speedup vs baseline: 1.1343x; 1.0406x over previous
"""GPT2 paged-attention decode kernel for Trainium2 (Bass/Tile), 8-core SPMD.

Problem: B=32 batches, Q=1 query, D=1024, H=16 heads, DH=64, KV cache 8192.
  qkv = hidden @ w_attn + b_attn; split into q, k_cur, v_cur
  attention over concat(cache, current) per (b, h)  [no scaling, no mask]
  out = (softmax contexts) @ w_proj + b_proj

Sharding: pure batch data-parallel, 4 batches per core, zero collectives.

Key optimization vs the fp32 version: K/V caches (and weights/activations)
are cast to fp16 on the HOST, halving device HBM traffic (256MB -> 128MB per
core) and unlocking the fast 16-bit paths on every engine:
  - DVE tensor_tensor runs in 2x_1p mode (0.5 cyc/elem)
  - TensorE matmul streams 1 col/cycle (fp32 is 4 cyc/col)
Precision: products/partial sums in fp16 give |err(s)| ~ 5e-3 on scores of
magnitude ~20 -> ~0.5% weight error, far inside the 2e-2 gate.

Softmax max-subtraction is dropped (shift invariance; fp32/bf16 exp cannot
overflow for |s| <~ 30: e^30 ~ 1e13 << bf16/fp32 max). exp outputs are
stored bf16 (range to 3e38) for the fast matmul path; denominators are
accumulated in fp32 via the activation accum_out.

Per (b,h) pair on-chip schedule:
  - K/V fp16 tiles [128, 64, 64] DMA'd in natural row layout (partition p
    holds keys p*64..p*64+63 contiguously; 1MB per DMA).
  - VectorE: kq = K * q_broadcast (fp16 2x); 3-level binary-tree adds over
    the head dim (fp16 2x); final 8->1 reduce_sum to fp32 s [128, 64].
  - ScalarE: e = exp(s) -> bf16 with accum_out giving per-partition
    denominator partials (fp32).
  - TensorE: ctx row [1,64] accumulated in PSUM via 64 matmuls, e columns
    stationary (1-col LDWEIGHTS), V tiles streaming; denominators reduced
    across partitions with a ones-matmul into a shared [1,64] PSUM row;
    ctx rows transposed into columns of ctx^T via rank-1 matmuls.
  - The current token never enters the stream: it is folded in a small
    epilogue over all 64 pairs at once (PE transposes of the qkv rows,
    one elementwise mul + ones-matmul partition reduce + exp).
"""

import os
import sys

import numpy as np

sys.path.insert(0, "/opt/trn_rl_repo")

import concourse.bass as bass
import concourse.tile as tile
from concourse import bacc, mybir
from concourse.bass_utils import run_bass_kernel_spmd

FP32 = mybir.dt.float32
FP16 = mybir.dt.float16
BF16 = mybir.dt.bfloat16

# Problem shape (hardcoded per contest rules).
B, D, H, DH, KV = 32, 1024, 16, 64, 8192
NCORES = 8
BC = B // NCORES          # batches per core = 4
NPAIRS = BC * H           # 64 (b,h) pairs per core
JT = KV // 128            # 64 key-slots per partition


def _bcast_ap(t_ap, col0, ncols, nparts, row_stride):
    """DRAM AP [nparts, rows, ncols] with partition stride 0 (broadcast)."""
    return bass.AP(
        tensor=t_ap.tensor,
        offset=t_ap.offset + col0,
        ap=[[0, nparts], [row_stride, t_ap.shape[0]], [1, ncols]],
    )


def build_kernel():
    nc = bacc.Bacc(
        "TRN2",
        target_bir_lowering=False,
        debug=False,
        enable_asserts=False,
        num_devices=NCORES,
    )

    hidden = nc.dram_tensor("hidden", [BC, D], FP16, kind="ExternalInput")
    k_cache = nc.dram_tensor("k_cache", [BC, H, KV, DH], FP16, kind="ExternalInput")
    v_cache = nc.dram_tensor("v_cache", [BC, H, KV, DH], FP16, kind="ExternalInput")
    # host-prepped: wa_prep[nb, p, t, n] = w_attn[t*128+p, nb*512+n]  (fp16)
    w_attn = nc.dram_tensor("w_attn", [6, 128, 8, 512], FP16, kind="ExternalInput")
    b_attn = nc.dram_tensor("b_attn", [3 * D], FP32, kind="ExternalInput")
    # host-prepped: wp_prep[p, h, n] = w_proj[h*64+p, n]  (fp16)
    w_proj = nc.dram_tensor("w_proj", [64, H, D], FP16, kind="ExternalInput")
    b_proj = nc.dram_tensor("b_proj", [D], FP32, kind="ExternalInput")
    ident = nc.dram_tensor("ident", [128, 128], FP16, kind="ExternalInput")
    selector = nc.dram_tensor("selector", [BC, BC, 128], FP16, kind="ExternalInput")
    out = nc.dram_tensor("out", [BC, D], FP32, kind="ExternalOutput")

    with tile.TileContext(nc) as tc:
        with (
            tc.tile_pool(name="singles", bufs=1) as singles,
            tc.tile_pool(name="wa_pool", bufs=2) as wa_pool,
            tc.tile_pool(name="k_pool", bufs=4) as k_pool,
            tc.tile_pool(name="v_pool", bufs=3) as v_pool,
            tc.tile_pool(name="wp_pool", bufs=2) as wp_pool,
            tc.tile_pool(name="se_pool", bufs=4) as se_pool,
            tc.tile_pool(name="mm512", bufs=2, space="PSUM") as mm512,
            tc.tile_pool(name="ctx_ps", bufs=2, space="PSUM") as ctx_ps,
            tc.tile_pool(name="ctxt_ps", bufs=2, space="PSUM") as ctxt_ps,
            tc.tile_pool(name="po_ps", bufs=2, space="PSUM") as po_ps,
        ):
            # ---- constants ----
            ones32 = singles.tile([128, 64], FP32)
            nc.vector.memset(ones32, 1.0)
            ones16 = singles.tile([128, 64], FP16)
            nc.vector.memset(ones16, 1.0)
            id_sb = singles.tile([128, 128], FP16)
            nc.gpsimd.dma_start(out=id_sb, in_=ident.ap())

            ba_sb = singles.tile([BC, 3 * D], FP32)
            nc.gpsimd.dma_start(
                out=ba_sb, in_=_bcast_ap(b_attn.ap().unsqueeze(0), 0, 3 * D, BC, 0)
            )
            bp_sb = singles.tile([BC, D], FP32)
            nc.gpsimd.dma_start(
                out=bp_sb, in_=_bcast_ap(b_proj.ap().unsqueeze(0), 0, D, BC, 0)
            )
            # batch-selector rows: sel[:, b, :] is [4, 128] with row b all-ones
            # (used as matmul lhsT to broadcast q row b across 128 partitions)
            sel = singles.tile([BC, BC, 128], FP16)
            nc.gpsimd.dma_start(out=sel, in_=selector.ap())

            # ---- hidden -> hT via PE transposes ----
            hid_sb = singles.tile([BC, D], FP16)
            nc.gpsimd.dma_start(out=hid_sb, in_=hidden.ap())
            ps_hT = mm512.tile([128, 8, BC], FP16, tag="mm512", name="ps_hT")
            for t in range(8):
                nc.tensor.transpose(
                    ps_hT[:, t, :],
                    hid_sb[:, t * 128 : (t + 1) * 128],
                    id_sb[0:BC, 0:BC],
                )
            hT = singles.tile([128, 8, BC], FP16)
            nc.scalar.copy(hT, ps_hT)

            # ---- QKV projection: qkv = hidden @ w_attn + b_attn ----
            # q third first (unblocks the attention stream), then k/v thirds.
            qkv_sb = singles.tile([BC, 3 * D], FP16)
            qb_all = singles.tile([128, BC, D], FP16)

            def qkv_block(nb):
                wa_t = wa_pool.tile([128, 8, 512], FP16, tag="wa")
                nc.gpsimd.dma_start(out=wa_t, in_=w_attn.ap()[nb])
                ps = mm512.tile([BC, 512], FP32, tag="mm512")
                for t in range(8):
                    nc.tensor.matmul(
                        ps,
                        hT[:, t, :],
                        wa_t[:, t, :],
                        start=(t == 0),
                        stop=(t == 7),
                    )
                nc.vector.tensor_add(
                    qkv_sb[:, nb * 512 : (nb + 1) * 512],
                    ps,
                    ba_sb[:, nb * 512 : (nb + 1) * 512],
                )
                if nb in (0, 1):
                    # qb_all[p, b, :] = q row of batch b on every partition:
                    # rank-K=4 matmuls with one-hot-row selectors (stays off
                    # the DMA queues entirely)
                    for b in range(BC):
                        ps_qb = mm512.tile(
                            [128, 512], FP32, tag="mm512", name=f"ps_qb{nb}_{b}"
                        )
                        nc.tensor.matmul(
                            ps_qb,
                            sel[:, b, :],
                            qkv_sb[:, nb * 512 : (nb + 1) * 512],
                            start=True,
                            stop=True,
                        )
                        nc.scalar.copy(
                            qb_all[:, b, nb * 512 : (nb + 1) * 512], ps_qb
                        )

            # only the q third now — the k_cur/v_cur thirds are emitted
            # after the stream has started (they'd head-of-line-block the
            # stream's DVE/PE work otherwise)
            qkv_block(0)
            qkv_block(1)

            # current-token epilogue tiles (filled by cur_token_block below,
            # emitted once the stream is in flight)
            e_cur = singles.tile([1, NPAIRS], FP32)
            vc = singles.tile([64, NPAIRS], FP32)

            def cur_token_block():
                # Transpose q/k_cur/v_cur rows [4, 64] -> [64, (h, b)] tiles.
                ps_tr = mm512.tile([64, 3, H, BC], FP16, tag="mm512", name="ps_tr")
                for part in range(3):
                    for h in range(H):
                        nc.tensor.transpose(
                            ps_tr[:, part, h, :],
                            qkv_sb[:, part * D + h * DH : part * D + (h + 1) * DH],
                            id_sb[0:BC, 0:BC],
                        )
                qT = singles.tile([64, H, BC], FP16)
                nc.scalar.copy(qT, ps_tr[:, 0])
                kT = singles.tile([64, H, BC], FP16)
                nc.scalar.copy(kT, ps_tr[:, 1])
                vT = singles.tile([64, H, BC], FP32)
                nc.vector.tensor_copy(vT, ps_tr[:, 2])

                # s_cur[j] = sum_d q[d, j]*k_cur[d, j] (partition reduce by MM)
                qk = singles.tile([64, H, BC], FP16)
                nc.vector.tensor_mul(qk, qT, kT)
                ps_scur = mm512.tile([1, NPAIRS], FP32, tag="mm512", name="ps_scur")
                nc.tensor.matmul(
                    ps_scur, ones16[0:64, 0:1], qk, start=True, stop=True
                )
                nc.scalar.activation(
                    e_cur, ps_scur, mybir.ActivationFunctionType.Exp
                )
                # broadcast e_cur across 64 partitions (ones outer product)
                ps_eb = mm512.tile([64, NPAIRS], FP32, tag="mm512", name="ps_eb")
                nc.tensor.matmul(
                    ps_eb, ones32[0:1, 0:64], e_cur, start=True, stop=True
                )
                eb_sb = singles.tile([64, NPAIRS], FP32)
                nc.scalar.copy(eb_sb, ps_eb)
                nc.vector.tensor_mul(vc, vT, eb_sb)

            # ---- attention stream over pairs (h-major: j = h*BC + b) ----
            # c_proj accumulators live across the whole stream (filled
            # incrementally as each head pair completes)
            ps_o = [
                po_ps.tile([BC, 512], FP32, tag="po", name=f"ps_o{i}")
                for i in range(2)
            ]
            ctxT_hh = None  # per-head-pair [65, 8] tile: rows 0-63 = ctx^T
            # columns for (i, b); row 64 = softmax denominators

            # Two (b, h) pairs per group (adjacent heads of one batch): DVE
            # ops process both pairs' tiles at once to amortize per-op cost.
            for g in range(NPAIRS // 2):
                hh, b = g // BC, g % BC
                h0 = 2 * hh

                if g == 2:
                    # stream is rolling: now emit the k_cur/v_cur qkv thirds
                    # and the current-token preparation
                    for nb in (2, 3, 4, 5):
                        qkv_block(nb)
                    cur_token_block()

                if b == 0:
                    ctxT_hh = ctxt_ps.tile(
                        [65, 2 * BC], FP32, tag="ctxt", name=f"ctxT{hh}"
                    )
                    # prefetch this head pair's w_proj rows (used at b == 3)
                    wp_t = wp_pool.tile([64, 2, D], FP16, tag="wp")
                    nc.gpsimd.dma_start(out=wp_t, in_=w_proj.ap()[:, h0 : h0 + 2, :])

                k2 = k_pool.tile([128, 2, JT, DH], FP16, tag="k")
                v2 = v_pool.tile([128, 2, JT, DH], FP16, tag="v")
                for i in range(2):
                    nc.sync.dma_start(
                        out=k2[:, i],
                        in_=k_cache.ap()[b, h0 + i].rearrange(
                            "(p jj) d -> p jj d", jj=JT
                        ),
                    )
                    nc.scalar.dma_start(
                        out=v2[:, i],
                        in_=v_cache.ap()[b, h0 + i].rearrange(
                            "(p jj) d -> p jj d", jj=JT
                        ),
                    )

                # s[p, hh, jj] = sum_d K[p, hh, jj, d] * q_hh[d]
                # fp16 2x mul + binary-tree halving adds, all in-place in k2
                # (K is dead after the mul; saves a whole SBUF pool)
                qb = (
                    qb_all[:, b, h0 * DH : (h0 + 2) * DH]
                    .rearrange("p (i d) -> p i d", i=2)
                    .unsqueeze(2)
                    .broadcast_to([128, 2, JT, DH])
                )
                nc.vector.tensor_mul(k2, k2, qb)
                nc.vector.tensor_add(
                    k2[:, :, :, 0:32], k2[:, :, :, 0:32], k2[:, :, :, 32:64]
                )
                nc.vector.tensor_add(
                    k2[:, :, :, 0:16], k2[:, :, :, 0:16], k2[:, :, :, 16:32]
                )
                nc.vector.tensor_add(
                    k2[:, :, :, 0:8], k2[:, :, :, 0:8], k2[:, :, :, 8:16]
                )
                s2 = se_pool.tile([128, 2, JT], FP32, tag="s")
                nc.vector.reduce_sum(s2, k2[:, :, :, 0:8], axis=mybir.AxisListType.X)

                # per pair: exp (bf16) + denominator partials, ctx matmuls
                e2 = se_pool.tile([128, 2, JT], BF16, tag="e")
                for i in range(2):
                    col = i * BC + b
                    lp = se_pool.tile([128, 1], FP32, tag="lp")
                    nc.scalar.activation(
                        e2[:, i],
                        s2[:, i],
                        mybir.ActivationFunctionType.Exp,
                        accum_out=lp,
                    )
                    psum_row = ctx_ps.tile([1, DH], FP32, tag="ctx")
                    for jj in range(JT):
                        nc.tensor.matmul(
                            psum_row,
                            e2[:, i, jj : jj + 1],
                            v2[:, i, jj, :],
                            start=(jj == 0),
                            stop=(jj == JT - 1),
                        )
                    # denominator into row 64 of the head-pair tile
                    nc.tensor.matmul(
                        ctxT_hh[64:65, col : col + 1],
                        lp,
                        ones32[:, 0:1],
                        start=True,
                        stop=True,
                        skip_group_check=True,
                    )
                    # transpose the row into the (i, b) column of ctx^T
                    ctx_row = se_pool.tile([1, DH], FP32, tag="ctxrow")
                    nc.scalar.copy(ctx_row, psum_row)
                    nc.tensor.matmul(
                        ctxT_hh[0:64, col : col + 1],
                        ctx_row,
                        ones32[0:1, 0:1],
                        start=True,
                        stop=True,
                        skip_group_check=True,
                    )

                if b == BC - 1:
                    # Head pair hh complete for all batches: fold in the
                    # current token, normalize, and run its c_proj matmuls
                    # now (keeps the post-stream tail tiny).
                    sl = slice(h0 * BC, (h0 + 2) * BC)
                    lt = se_pool.tile([1, 2 * BC], FP32, tag="lt")
                    nc.vector.tensor_add(lt, ctxT_hh[64:65, :], e_cur[:, sl])
                    rr = se_pool.tile([1, 2 * BC], FP32, tag="rr")
                    nc.vector.reciprocal(rr, lt)
                    ps_rb = ctx_ps.tile(
                        [64, 2 * BC], FP32, tag="ctx", name=f"ps_rb{hh}"
                    )
                    nc.tensor.matmul(
                        ps_rb,
                        ones32[0:1, 0:64],
                        rr,
                        start=True,
                        stop=True,
                        skip_group_check=True,
                    )
                    rb8 = se_pool.tile([64, 2 * BC], FP32, tag="rb8")
                    nc.scalar.copy(rb8, ps_rb)
                    cf8 = se_pool.tile([64, 2 * BC], FP32, tag="cf8")
                    nc.vector.tensor_add(cf8, ctxT_hh[0:64, :], vc[:, sl])
                    cs8 = se_pool.tile([64, 2 * BC], FP16, tag="cs8")
                    nc.vector.tensor_mul(cs8, cf8, rb8)
                    for i in range(2):
                        for nb in range(2):
                            nc.tensor.matmul(
                                ps_o[nb],
                                cs8[:, i * BC : (i + 1) * BC],
                                wp_t[:, i, nb * 512 : (nb + 1) * 512],
                                start=(h0 + i == 0),
                                stop=(h0 + i == H - 1),
                                skip_group_check=True,
                            )

            # ---- tail: output bias + store ----
            out_sb = singles.tile([BC, D], FP32)
            for nb in range(2):
                nc.vector.tensor_add(
                    out_sb[:, nb * 512 : (nb + 1) * 512],
                    ps_o[nb],
                    bp_sb[:, nb * 512 : (nb + 1) * 512],
                )
            nc.gpsimd.dma_start(out=out.ap(), in_=out_sb)

    nc.compile()
    return nc


_NC_CACHE = None


def _get_nc():
    global _NC_CACHE
    if _NC_CACHE is None:
        _NC_CACHE = build_kernel()
    return _NC_CACHE


def make_in_maps(inputs):
    """Shard full inputs into per-core input maps (batch data-parallel).

    Heavy tensors are cast to fp16 host-side: the device kernel streams
    fp16, halving HBM traffic.
    """
    hidden = np.asarray(inputs["hidden_states"], np.float32).reshape(B, D)
    hidden16 = np.ascontiguousarray(hidden.astype(np.float16))
    k16 = np.asarray(inputs["k_cache"]).astype(np.float16)
    v16 = np.asarray(inputs["v_cache"]).astype(np.float16)
    wa = np.asarray(inputs["w_attn"], np.float32).astype(np.float16)
    # wa_prep[nb, p, t, n] = w_attn[t*128+p, nb*512+n]
    wa_prep = np.ascontiguousarray(
        wa.reshape(8, 128, 6, 512).transpose(2, 1, 0, 3)
    )
    wp = np.asarray(inputs["w_proj"], np.float32).astype(np.float16)
    # wp_prep[p, h, n] = w_proj[h*64+p, n]
    wp_prep = np.ascontiguousarray(wp.reshape(H, 64, D).transpose(1, 0, 2))
    b_attn = np.ascontiguousarray(np.asarray(inputs["b_attn"], np.float32))
    b_proj = np.ascontiguousarray(np.asarray(inputs["b_proj"], np.float32))
    ident = np.eye(128, dtype=np.float16)
    selector = np.zeros((BC, BC, 128), np.float16)
    for b in range(BC):
        selector[b, b, :] = 1.0
    in_maps = []
    for c in range(NCORES):
        sl = slice(c * BC, (c + 1) * BC)
        in_maps.append(
            {
                "hidden": np.ascontiguousarray(hidden16[sl]),
                "k_cache": np.ascontiguousarray(k16[sl]),
                "v_cache": np.ascontiguousarray(v16[sl]),
                "w_attn": wa_prep,
                "b_attn": b_attn,
                "w_proj": wp_prep,
                "b_proj": b_proj,
                "ident": ident,
                "selector": selector,
            }
        )
    return in_maps


def kernel(**inputs):
    nc = _get_nc()
    in_maps = make_in_maps(inputs)
    res = run_bass_kernel_spmd(nc, in_maps, core_ids=list(range(NCORES)))
    out = np.empty((B, 1, D), np.float32)
    for c in range(NCORES):
        out[c * BC : (c + 1) * BC, 0, :] = np.asarray(res.results[c]["out"])
    return out


if __name__ == "__main__":
    build_kernel()
    print("kernel built OK")


# revision 45
# speedup vs baseline: 1.1370x; 1.0023x over previous
"""GPT2 paged-attention decode kernel for Trainium2 (Bass/Tile), 8-core SPMD.

Problem: B=32 batches, Q=1 query, D=1024, H=16 heads, DH=64, KV cache 8192.
  qkv = hidden @ w_attn + b_attn; split into q, k_cur, v_cur
  attention over concat(cache, current) per (b, h)  [no scaling, no mask]
  out = (softmax contexts) @ w_proj + b_proj

Sharding: pure batch data-parallel, 4 batches per core, zero collectives.

Key optimization vs the fp32 version: K/V caches (and weights/activations)
are cast to fp16 on the HOST, halving device HBM traffic (256MB -> 128MB per
core) and unlocking the fast 16-bit paths on every engine:
  - DVE tensor_tensor runs in 2x_1p mode (0.5 cyc/elem)
  - TensorE matmul streams 1 col/cycle (fp32 is 4 cyc/col)
Precision: products/partial sums in fp16 give |err(s)| ~ 5e-3 on scores of
magnitude ~20 -> ~0.5% weight error, far inside the 2e-2 gate.

Softmax max-subtraction is dropped (shift invariance; fp32/bf16 exp cannot
overflow for |s| <~ 30: e^30 ~ 1e13 << bf16/fp32 max). exp outputs are
stored bf16 (range to 3e38) for the fast matmul path; denominators are
accumulated in fp32 via the activation accum_out.

Per (b,h) pair on-chip schedule:
  - K/V fp16 tiles [128, 64, 64] DMA'd in natural row layout (partition p
    holds keys p*64..p*64+63 contiguously; 1MB per DMA).
  - VectorE: kq = K * q_broadcast (fp16 2x); 3-level binary-tree adds over
    the head dim (fp16 2x); final 8->1 reduce_sum to fp32 s [128, 64].
  - ScalarE: e = exp(s) -> bf16 with accum_out giving per-partition
    denominator partials (fp32).
  - TensorE: ctx row [1,64] accumulated in PSUM via 64 matmuls, e columns
    stationary (1-col LDWEIGHTS), V tiles streaming; denominators reduced
    across partitions with a ones-matmul into a shared [1,64] PSUM row;
    ctx rows transposed into columns of ctx^T via rank-1 matmuls.
  - The current token never enters the stream: it is folded in a small
    epilogue over all 64 pairs at once (PE transposes of the qkv rows,
    one elementwise mul + ones-matmul partition reduce + exp).
"""

import os
import sys

import numpy as np

sys.path.insert(0, "/opt/trn_rl_repo")

import concourse.bass as bass
import concourse.tile as tile
from concourse import bacc, mybir
from concourse.bass_utils import run_bass_kernel_spmd

FP32 = mybir.dt.float32
FP16 = mybir.dt.float16
BF16 = mybir.dt.bfloat16

# Problem shape (hardcoded per contest rules).
B, D, H, DH, KV = 32, 1024, 16, 64, 8192
NCORES = 8
BC = B // NCORES          # batches per core = 4
NPAIRS = BC * H           # 64 (b,h) pairs per core
JT = KV // 128            # 64 key-slots per partition


def _bcast_ap(t_ap, col0, ncols, nparts, row_stride):
    """DRAM AP [nparts, rows, ncols] with partition stride 0 (broadcast)."""
    return bass.AP(
        tensor=t_ap.tensor,
        offset=t_ap.offset + col0,
        ap=[[0, nparts], [row_stride, t_ap.shape[0]], [1, ncols]],
    )


def build_kernel():
    nc = bacc.Bacc(
        "TRN2",
        target_bir_lowering=False,
        debug=False,
        enable_asserts=False,
        num_devices=NCORES,
    )

    hidden = nc.dram_tensor("hidden", [BC, D], FP16, kind="ExternalInput")
    k_cache = nc.dram_tensor("k_cache", [BC, H, KV, DH], FP16, kind="ExternalInput")
    v_cache = nc.dram_tensor("v_cache", [BC, H, KV, DH], FP16, kind="ExternalInput")
    # host-prepped: wa_prep[nb, p, t, n] = w_attn[t*128+p, nb*512+n]  (fp16)
    w_attn = nc.dram_tensor("w_attn", [6, 128, 8, 512], FP16, kind="ExternalInput")
    b_attn = nc.dram_tensor("b_attn", [3 * D], FP32, kind="ExternalInput")
    # host-prepped: wp_prep[p, h, n] = w_proj[h*64+p, n]  (fp16)
    w_proj = nc.dram_tensor("w_proj", [64, H, D], FP16, kind="ExternalInput")
    b_proj = nc.dram_tensor("b_proj", [D], FP32, kind="ExternalInput")
    ident = nc.dram_tensor("ident", [128, 128], FP16, kind="ExternalInput")
    selector = nc.dram_tensor("selector", [BC, BC, 128], FP16, kind="ExternalInput")
    out = nc.dram_tensor("out", [BC, D], FP32, kind="ExternalOutput")

    with tile.TileContext(nc) as tc:
        with (
            tc.tile_pool(name="singles", bufs=1) as singles,
            tc.tile_pool(name="wa_pool", bufs=1) as wa_pool,
            tc.tile_pool(name="k_pool", bufs=4) as k_pool,
            tc.tile_pool(name="v_pool", bufs=4) as v_pool,
            tc.tile_pool(name="wp_pool", bufs=2) as wp_pool,
            tc.tile_pool(name="se_pool", bufs=4) as se_pool,
            tc.tile_pool(name="mm512", bufs=2, space="PSUM") as mm512,
            tc.tile_pool(name="ctx_ps", bufs=2, space="PSUM") as ctx_ps,
            tc.tile_pool(name="ctxt_ps", bufs=2, space="PSUM") as ctxt_ps,
            tc.tile_pool(name="po_ps", bufs=2, space="PSUM") as po_ps,
        ):
            # ---- constants ----
            ones32 = singles.tile([128, 64], FP32)
            nc.vector.memset(ones32, 1.0)
            ones16 = singles.tile([128, 64], FP16)
            nc.vector.memset(ones16, 1.0)
            id_sb = singles.tile([128, 128], FP16)
            nc.gpsimd.dma_start(out=id_sb, in_=ident.ap())

            ba_sb = singles.tile([BC, 3 * D], FP16)
            nc.gpsimd.dma_start(
                out=ba_sb, in_=_bcast_ap(b_attn.ap().unsqueeze(0), 0, 3 * D, BC, 0)
            )
            bp_sb = singles.tile([BC, D], FP32)
            nc.gpsimd.dma_start(
                out=bp_sb, in_=_bcast_ap(b_proj.ap().unsqueeze(0), 0, D, BC, 0)
            )
            # batch-selector rows: sel[:, b, :] is [4, 128] with row b all-ones
            # (used as matmul lhsT to broadcast q row b across 128 partitions)
            sel = singles.tile([BC, BC, 128], FP16)
            nc.gpsimd.dma_start(out=sel, in_=selector.ap())

            # ---- hidden -> hT via PE transposes ----
            hid_sb = singles.tile([BC, D], FP16)
            nc.gpsimd.dma_start(out=hid_sb, in_=hidden.ap())
            ps_hT = mm512.tile([128, 8, BC], FP16, tag="mm512", name="ps_hT")
            for t in range(8):
                nc.tensor.transpose(
                    ps_hT[:, t, :],
                    hid_sb[:, t * 128 : (t + 1) * 128],
                    id_sb[0:BC, 0:BC],
                )
            hT = singles.tile([128, 8, BC], FP16)
            nc.scalar.copy(hT, ps_hT)

            # ---- QKV projection: qkv = hidden @ w_attn + b_attn ----
            # q third first (unblocks the attention stream), then k/v thirds.
            qkv_sb = singles.tile([BC, 3 * D], FP16)
            qb_all = singles.tile([128, BC, D], FP16)

            def qkv_block(nb):
                wa_t = wa_pool.tile([128, 8, 512], FP16, tag="wa")
                nc.gpsimd.dma_start(out=wa_t, in_=w_attn.ap()[nb])
                ps = mm512.tile([BC, 512], FP32, tag="mm512")
                for t in range(8):
                    nc.tensor.matmul(
                        ps,
                        hT[:, t, :],
                        wa_t[:, t, :],
                        start=(t == 0),
                        stop=(t == 7),
                    )
                nc.vector.tensor_add(
                    qkv_sb[:, nb * 512 : (nb + 1) * 512],
                    ps,
                    ba_sb[:, nb * 512 : (nb + 1) * 512],
                )
                if nb in (0, 1):
                    # qb_all[p, b, :] = q row of batch b on every partition:
                    # rank-K=4 matmuls with one-hot-row selectors (stays off
                    # the DMA queues entirely)
                    for b in range(BC):
                        ps_qb = mm512.tile(
                            [128, 512], FP32, tag="mm512", name=f"ps_qb{nb}_{b}"
                        )
                        nc.tensor.matmul(
                            ps_qb,
                            sel[:, b, :],
                            qkv_sb[:, nb * 512 : (nb + 1) * 512],
                            start=True,
                            stop=True,
                        )
                        nc.scalar.copy(
                            qb_all[:, b, nb * 512 : (nb + 1) * 512], ps_qb
                        )

            # only the q third now — the k_cur/v_cur thirds are emitted
            # after the stream has started (they'd head-of-line-block the
            # stream's DVE/PE work otherwise)
            qkv_block(0)
            qkv_block(1)

            # current-token epilogue tiles (filled by cur_token_block below,
            # emitted once the stream is in flight)
            e_cur = singles.tile([1, NPAIRS], FP32)
            vc = singles.tile([64, NPAIRS], FP32)

            def cur_token_block():
                # Transpose q/k_cur/v_cur rows [4, 64] -> [64, (h, b)] tiles.
                ps_tr = mm512.tile([64, 3, H, BC], FP16, tag="mm512", name="ps_tr")
                for part in range(3):
                    for h in range(H):
                        nc.tensor.transpose(
                            ps_tr[:, part, h, :],
                            qkv_sb[:, part * D + h * DH : part * D + (h + 1) * DH],
                            id_sb[0:BC, 0:BC],
                        )
                qT = singles.tile([64, H, BC], FP16)
                nc.scalar.copy(qT, ps_tr[:, 0])
                kT = singles.tile([64, H, BC], FP16)
                nc.scalar.copy(kT, ps_tr[:, 1])
                vT = singles.tile([64, H, BC], FP32)
                nc.vector.tensor_copy(vT, ps_tr[:, 2])

                # s_cur[j] = sum_d q[d, j]*k_cur[d, j] (partition reduce by MM)
                qk = singles.tile([64, H, BC], FP16)
                nc.vector.tensor_mul(qk, qT, kT)
                ps_scur = mm512.tile([1, NPAIRS], FP32, tag="mm512", name="ps_scur")
                nc.tensor.matmul(
                    ps_scur, ones16[0:64, 0:1], qk, start=True, stop=True
                )
                nc.scalar.activation(
                    e_cur, ps_scur, mybir.ActivationFunctionType.Exp
                )
                # broadcast e_cur across 64 partitions (ones outer product)
                ps_eb = mm512.tile([64, NPAIRS], FP32, tag="mm512", name="ps_eb")
                nc.tensor.matmul(
                    ps_eb, ones32[0:1, 0:64], e_cur, start=True, stop=True
                )
                eb_sb = singles.tile([64, NPAIRS], FP32)
                nc.scalar.copy(eb_sb, ps_eb)
                nc.vector.tensor_mul(vc, vT, eb_sb)

            # ---- attention stream over pairs (h-major: j = h*BC + b) ----
            # c_proj accumulators live across the whole stream (filled
            # incrementally as each head pair completes)
            ps_o = [
                po_ps.tile([BC, 512], FP32, tag="po", name=f"ps_o{i}")
                for i in range(2)
            ]
            ctxT_hh = None  # per-head-pair [65, 8] tile: rows 0-63 = ctx^T
            # columns for (i, b); row 64 = softmax denominators

            # Two (b, h) pairs per group (adjacent heads of one batch): DVE
            # ops process both pairs' tiles at once to amortize per-op cost.
            for g in range(NPAIRS // 2):
                hh, b = g // BC, g % BC
                h0 = 2 * hh

                if g == 2:
                    # stream is rolling: now emit the k_cur/v_cur qkv thirds
                    # and the current-token preparation
                    for nb in (2, 3, 4, 5):
                        qkv_block(nb)
                    cur_token_block()

                if b == 0:
                    ctxT_hh = ctxt_ps.tile(
                        [65, 2 * BC], FP32, tag="ctxt", name=f"ctxT{hh}"
                    )
                    # prefetch this head pair's w_proj rows (used at b == 3)
                    wp_t = wp_pool.tile([64, 2, D], FP16, tag="wp")
                    nc.gpsimd.dma_start(out=wp_t, in_=w_proj.ap()[:, h0 : h0 + 2, :])

                k2 = k_pool.tile([128, 2, JT, DH], FP16, tag="k")
                v2 = v_pool.tile([128, 2, JT, DH], FP16, tag="v")
                nc.sync.dma_start(
                    out=k2,
                    in_=k_cache.ap()[b, h0 : h0 + 2].rearrange(
                        "i (p jj) d -> p i jj d", jj=JT
                    ),
                )
                nc.scalar.dma_start(
                    out=v2,
                    in_=v_cache.ap()[b, h0 : h0 + 2].rearrange(
                        "i (p jj) d -> p i jj d", jj=JT
                    ),
                )

                # s[p, hh, jj] = sum_d K[p, hh, jj, d] * q_hh[d]
                # fp16 2x mul + binary-tree halving adds, all in-place in k2
                # (K is dead after the mul; saves a whole SBUF pool)
                qb = (
                    qb_all[:, b, h0 * DH : (h0 + 2) * DH]
                    .rearrange("p (i d) -> p i d", i=2)
                    .unsqueeze(2)
                    .broadcast_to([128, 2, JT, DH])
                )
                nc.vector.tensor_mul(k2, k2, qb)
                nc.vector.tensor_add(
                    k2[:, :, :, 0:32], k2[:, :, :, 0:32], k2[:, :, :, 32:64]
                )
                nc.vector.tensor_add(
                    k2[:, :, :, 0:16], k2[:, :, :, 0:16], k2[:, :, :, 16:32]
                )
                nc.vector.tensor_add(
                    k2[:, :, :, 0:8], k2[:, :, :, 0:8], k2[:, :, :, 8:16]
                )
                s2 = se_pool.tile([128, 2, JT], FP32, tag="s")
                nc.vector.reduce_sum(s2, k2[:, :, :, 0:8], axis=mybir.AxisListType.X)

                # per pair: exp (bf16) + denominator partials, ctx matmuls
                e2 = se_pool.tile([128, 2, JT], BF16, tag="e")
                for i in range(2):
                    col = i * BC + b
                    lp = se_pool.tile([128, 1], FP32, tag="lp")
                    nc.scalar.activation(
                        e2[:, i],
                        s2[:, i],
                        mybir.ActivationFunctionType.Exp,
                        accum_out=lp,
                    )
                    psum_row = ctx_ps.tile([1, DH], FP32, tag="ctx")
                    for jj in range(JT):
                        nc.tensor.matmul(
                            psum_row,
                            e2[:, i, jj : jj + 1],
                            v2[:, i, jj, :],
                            start=(jj == 0),
                            stop=(jj == JT - 1),
                        )
                    # denominator into row 64 of the head-pair tile
                    nc.tensor.matmul(
                        ctxT_hh[64:65, col : col + 1],
                        lp,
                        ones32[:, 0:1],
                        start=True,
                        stop=True,
                        skip_group_check=True,
                    )
                    # transpose the row into the (i, b) column of ctx^T
                    ctx_row = se_pool.tile([1, DH], FP32, tag="ctxrow")
                    nc.scalar.copy(ctx_row, psum_row)
                    nc.tensor.matmul(
                        ctxT_hh[0:64, col : col + 1],
                        ctx_row,
                        ones32[0:1, 0:1],
                        start=True,
                        stop=True,
                        skip_group_check=True,
                    )

                if b == BC - 1:
                    # Head pair hh complete for all batches: fold in the
                    # current token, normalize, and run its c_proj matmuls
                    # now (keeps the post-stream tail tiny).
                    sl = slice(h0 * BC, (h0 + 2) * BC)
                    lt = se_pool.tile([1, 2 * BC], FP32, tag="lt")
                    nc.vector.tensor_add(lt, ctxT_hh[64:65, :], e_cur[:, sl])
                    rr = se_pool.tile([1, 2 * BC], FP32, tag="rr")
                    nc.vector.reciprocal(rr, lt)
                    ps_rb = ctx_ps.tile(
                        [64, 2 * BC], FP32, tag="ctx", name=f"ps_rb{hh}"
                    )
                    nc.tensor.matmul(
                        ps_rb,
                        ones32[0:1, 0:64],
                        rr,
                        start=True,
                        stop=True,
                        skip_group_check=True,
                    )
                    rb8 = se_pool.tile([64, 2 * BC], FP32, tag="rb8")
                    nc.scalar.copy(rb8, ps_rb)
                    cf8 = se_pool.tile([64, 2 * BC], FP32, tag="cf8")
                    nc.vector.tensor_add(cf8, ctxT_hh[0:64, :], vc[:, sl])
                    cs8 = se_pool.tile([64, 2 * BC], FP16, tag="cs8")
                    nc.vector.tensor_mul(cs8, cf8, rb8)
                    for i in range(2):
                        for nb in range(2):
                            nc.tensor.matmul(
                                ps_o[nb],
                                cs8[:, i * BC : (i + 1) * BC],
                                wp_t[:, i, nb * 512 : (nb + 1) * 512],
                                start=(h0 + i == 0),
                                stop=(h0 + i == H - 1),
                                skip_group_check=True,
                            )

            # ---- tail: output bias + store ----
            out_sb = singles.tile([BC, D], FP32)
            for nb in range(2):
                nc.vector.tensor_add(
                    out_sb[:, nb * 512 : (nb + 1) * 512],
                    ps_o[nb],
                    bp_sb[:, nb * 512 : (nb + 1) * 512],
                )
            nc.gpsimd.dma_start(out=out.ap(), in_=out_sb)

    nc.compile()
    return nc


_NC_CACHE = None


def _get_nc():
    global _NC_CACHE
    if _NC_CACHE is None:
        _NC_CACHE = build_kernel()
    return _NC_CACHE


def make_in_maps(inputs):
    """Shard full inputs into per-core input maps (batch data-parallel).

    Heavy tensors are cast to fp16 host-side: the device kernel streams
    fp16, halving HBM traffic.
    """
    hidden = np.asarray(inputs["hidden_states"], np.float32).reshape(B, D)
    hidden16 = np.ascontiguousarray(hidden.astype(np.float16))
    k16 = np.asarray(inputs["k_cache"]).astype(np.float16)
    v16 = np.asarray(inputs["v_cache"]).astype(np.float16)
    wa = np.asarray(inputs["w_attn"], np.float32).astype(np.float16)
    # wa_prep[nb, p, t, n] = w_attn[t*128+p, nb*512+n]
    wa_prep = np.ascontiguousarray(
        wa.reshape(8, 128, 6, 512).transpose(2, 1, 0, 3)
    )
    wp = np.asarray(inputs["w_proj"], np.float32).astype(np.float16)
    # wp_prep[p, h, n] = w_proj[h*64+p, n]
    wp_prep = np.ascontiguousarray(wp.reshape(H, 64, D).transpose(1, 0, 2))
    b_attn = np.ascontiguousarray(np.asarray(inputs["b_attn"], np.float32))
    b_proj = np.ascontiguousarray(np.asarray(inputs["b_proj"], np.float32))
    ident = np.eye(128, dtype=np.float16)
    selector = np.zeros((BC, BC, 128), np.float16)
    for b in range(BC):
        selector[b, b, :] = 1.0
    in_maps = []
    for c in range(NCORES):
        sl = slice(c * BC, (c + 1) * BC)
        in_maps.append(
            {
                "hidden": np.ascontiguousarray(hidden16[sl]),
                "k_cache": np.ascontiguousarray(k16[sl]),
                "v_cache": np.ascontiguousarray(v16[sl]),
                "w_attn": wa_prep,
                "b_attn": b_attn,
                "w_proj": wp_prep,
                "b_proj": b_proj,
                "ident": ident,
                "selector": selector,
            }
        )
    return in_maps


def kernel(**inputs):
    nc = _get_nc()
    in_maps = make_in_maps(inputs)
    res = run_bass_kernel_spmd(nc, in_maps, core_ids=list(range(NCORES)))
    out = np.empty((B, 1, D), np.float32)
    for c in range(NCORES):
        out[c * BC : (c + 1) * BC, 0, :] = np.asarray(res.results[c]["out"])
    return out


if __name__ == "__main__":
    build_kernel()
    print("kernel built OK")
